# revision 12
# baseline (speedup 1.0000x reference)
"""Banded (Longformer-style) multi-head attention on 8 TRN2 NeuronCores.

Sharding: 16 heads are split 2-per-core (tensor parallel on H); every
core sees all 8192 tokens.  Compute dtype is bf16 (f32 accumulate in
PSUM); inputs are pre-cast/pre-TRANSPOSED on the host, so the x input
stream is a plain strided DMA copy.

Per-core kernel (single NEFF, fine-grained software-pipelined emission
keyed on the 128-wide key tile index so proj / attention / out-proj
interleave at ~2.5us granularity on the PE):
  1. DMA feature-major xT slabs DRAM->SBUF; project to qT,kT
     (feature-major [d, T]) via w-stationary matmuls and to v TOKEN-major
     via x-stationary matmuls (same FLOPs, no v transpose).  v is stored
     ones-augmented so the P@V matmul also produces the softmax
     denominator.  The v bias is folded into the output bias on the host
     (exact: attention rows sum to 1).
  2. j-major banded attention: for each 128-wide key tile j, one
     scores^T matmul [key,y x query-cols] against the <=5 query chunks
     in its band (K=64), exp on ScalarE without max-subtraction
     (scores are O(+-30), exact in f32), band-corner masking via
     affine_select on the GpSimd engine, then per-query-chunk
     P^T@V_aug accumulation (K=128) and a 1/den fixup on VectorE.
     The Scalar engine runs ONLY the exps so a score tile never queues
     behind unrelated copies (the score matmul's PSUM WAR on the exp is
     the tightest loop in the kernel).
  3. ctx 2-chunk groups transposed feature-major by the Ant DMA-transpose
     unit, dispatched 3+ chunks after their fixup so the SP queue never
     stalls holding the dispatch slot; partial output projection
     ctx_h @ Wo_h.T -> [8192, 1024] bf16.
The host sums the 8 partial outputs and adds the output bias (the
all-reduce step of tensor parallelism, done during the gather).
"""

import sys

sys.path.insert(0, "/opt/trn_rl_repo")

import numpy as np

import concourse.bass as bass
import concourse.mybir as mybir
import concourse.tile as tile
from concourse import bacc
from concourse.bass_utils import run_bass_kernel_spmd

F32 = mybir.dt.float32
BF16 = mybir.dt.bfloat16

B, S, D, E, H, HD = 2, 4096, 1024, 1024, 16, 64
W = 256                    # half window
T = B * S                  # 8192 flattened tokens
NCORES = 8
HPC = H // NCORES          # 2 heads per core
FQKV = 3 * HPC * HD        # 384 projected features per core
NT = T // 128              # 64 token chunks
CPS = S // 128             # 32 chunks per sequence
SLAB = 512                 # proj token slab
VROW = 2 * (HD + 1)        # 130: [v_h0(64) | 1 | v_h1(64) | 1]


def _build_program():
    nc = bacc.Bacc(None, target_bir_lowering=False, debug=False)

    xT_d = nc.dram_tensor("xbfT", [D, T], BF16, kind="ExternalInput")
    wqkvT_d = nc.dram_tensor("wqkvT", [D, FQKV], BF16, kind="ExternalInput")
    bqkv_d = nc.dram_tensor("bqkv", [FQKV], F32, kind="ExternalInput")
    woT_d = nc.dram_tensor("woT", [HPC * HD, E], BF16, kind="ExternalInput")
    out_d = nc.dram_tensor("out_p", [T, E], BF16, kind="ExternalOutput")

    with tile.TileContext(nc) as tc:
        with (
            tc.tile_pool(name="const", bufs=1) as cpool,
            tc.tile_pool(name="big", bufs=1) as bigpool,
            tc.tile_pool(name="xtp", bufs=4) as xtp,
            tc.tile_pool(name="cnp", bufs=4) as cnp,
            tc.tile_pool(name="recp", bufs=4) as recp,
            tc.tile_pool(name="ptp", bufs=10) as ptp,
            tc.tile_pool(name="outsb", bufs=2) as outsb,
            tc.tile_pool(name="ps512", bufs=3, space="PSUM") as ps512,
            tc.tile_pool(name="spsum", bufs=2, space="PSUM") as spsum,
            tc.tile_pool(name="cpsum", bufs=1, space="PSUM") as cpsum,
        ):
            # ---- constants (w_sb split per chunk so the first proj matmul
            # only waits on one 96KB piece) ----
            w_sb = cpool.tile([128, 8, FQKV], BF16, tag="w_sb")
            for c in range(8):
                nc.sync.dma_start(
                    w_sb[:, c, :], wqkvT_d[c * 128:(c + 1) * 128, :])
            b_sb = cpool.tile([128, 3], F32, tag="b_sb")
            nc.sync.dma_start(b_sb[:], bqkv_d[:].rearrange("(a p) -> p a", p=128))
            wo_sb = cpool.tile([128, E], BF16, tag="wo_sb")
            nc.sync.dma_start(wo_sb[:], woT_d[:])

            # ---- persistent activations ----
            q_sb = bigpool.tile([128, T], BF16, tag="q_sb")
            k_sb = bigpool.tile([128, T], BF16, tag="k_sb")
            v_sb = bigpool.tile([128, NT, VROW], BF16, tag="v_sb")
            ctxT_sb = bigpool.tile([128, T], BF16, tag="ctxT_sb")
            # ones columns of the augmented V (cols 64 and 129 of each chunk)
            nc.vector.memset(v_sb[:, :, HD::HD + 1], 1.0)

            # PV accumulators: 2 chunks x 2 heads packed in one PSUM bank
            ctx_ps = cpsum.tile([128, 4, HD + 1], F32, tag="ctx_ps")

            # ---- projection (split into DMA issue and compute) ----
            slabs = ([(0, 128), (128, 128), (256, 256)] +
                     [(512 * k, 512) for k in range(1, T // 512)])
            xT_tiles = {}

            def issue_xT(si):
                t0, wd = slabs[si]
                xT = xtp.tile([128, 8, SLAB], BF16, tag="xT")
                nc.sync.dma_start(
                    xT[:, :, 0:wd],
                    xT_d[:, t0:t0 + wd].rearrange("(c p) t -> p c t", p=128))
                xT_tiles[si] = xT

            def proj_qk(si):
                t0, ntok = slabs[si]
                xT = xT_tiles[si]
                # q, k: feature-major [128 feats, ntok]
                for ft in range(2):
                    ps = ps512.tile([128, SLAB], F32, tag="ps512")
                    for c in range(8):
                        nc.tensor.matmul(
                            ps[:, 0:ntok], w_sb[:, c, ft * 128:(ft + 1) * 128],
                            xT[:, c, 0:ntok], start=(c == 0), stop=(c == 7))
                    dest = (q_sb, k_sb)[ft]
                    nc.vector.tensor_scalar_add(
                        dest[:, t0:t0 + ntok], ps[:, 0:ntok], b_sb[:, ft:ft + 1])

            def proj_v(si):
                t0, ntok = slabs[si]
                nck = ntok // 128
                xT = xT_tiles.pop(si)
                # v: token-major [128 tokens, 128 feats] per chunk (no
                # transpose needed; v bias is folded into bo on the host)
                vps = ps512.tile([128, SLAB], F32, tag="ps512")
                for ck in range(nck):
                    for c in range(8):
                        nc.tensor.matmul(
                            vps[:, ck * 128:(ck + 1) * 128],
                            xT[:, c, ck * 128:(ck + 1) * 128],
                            w_sb[:, c, 2 * 128:3 * 128],
                            start=(c == 0), stop=(c == 7))
                for ck in range(nck):
                    gck = t0 // 128 + ck
                    nc.vector.tensor_copy(
                        v_sb[:, gck, :].rearrange(
                            "p (h r) -> p h r", h=2)[:, :, 0:HD],
                        vps[:, ck * 128:(ck + 1) * 128].rearrange(
                            "p (h r) -> p h r", h=2))

            # j-major scoresT: st_j[y, b*128:(b+1)*128] = k_j^T q_{c}, where
            # c = j-2+b.  pt_j = exp(st_j/8) with band corners zeroed via
            # affine_select on GpSimd.
            pt_tiles = {}
            cn_state = {}

            def scores_j(seq, j, h):
                b_lo = max(0, 2 - j)
                b_hi = min(4, 2 + (CPS - 1) - j)
                gj = seq * CPS + j
                st = spsum.tile([128, 640], F32, tag="st")
                lo, hi = b_lo * 128, (b_hi + 1) * 128
                qcols = (seq * CPS + j - 2) * 128
                pieces = [(a, b) for (a, b) in [(lo, min(hi, 512)), (512, hi)]
                          if b > a]
                for (a, b) in pieces:
                    nc.tensor.matmul(
                        st[:, a:b],
                        k_sb[h * HD:(h + 1) * HD, gj * 128:(gj + 1) * 128],
                        q_sb[h * HD:(h + 1) * HD, qcols + a:qcols + b],
                        start=True, stop=True)
                pt = ptp.tile([128, 640], BF16, tag="pt")
                nc.scalar.activation(
                    pt[:, lo:hi], st[:, lo:hi],
                    mybir.ActivationFunctionType.Exp,
                    scale=float(1.0 / np.sqrt(HD)))
                if b_lo == 0:
                    # b=0 <-> chunk c=j-2, m=4: keep y <= t  (p <= f)
                    nc.gpsimd.affine_select(
                        out=pt[:, 0:128], in_=pt[:, 0:128],
                        compare_op=mybir.AluOpType.is_ge, fill=0.0, base=0,
                        pattern=[[1, 128]], channel_multiplier=-1)
                if b_hi == 4:
                    # b=4 <-> chunk c=j+2, m=0: keep y >= t  (p >= f)
                    nc.gpsimd.affine_select(
                        out=pt[:, 512:640], in_=pt[:, 512:640],
                        compare_op=mybir.AluOpType.is_ge, fill=0.0, base=0,
                        pattern=[[-1, 128]], channel_multiplier=1)
                pt_tiles[(seq, j, h)] = pt

            def attention_chunk(gc):
                seq, c = divmod(gc, CPS)
                qi, ci = divmod(gc, 2)
                m_lo = max(0, 2 - c)
                m_hi = min(4, CPS - 1 - c + 2)
                nm = m_hi - m_lo + 1
                if ci == 0:
                    cn = cnp.tile([128, 2, 2, HD], BF16, tag="cn", name="cn")
                    cn_state[qi] = cn
                cn = cn_state[qi]
                for h in range(HPC):
                    ctx = ctx_ps[:, (gc % 2) * 2 + h, :]
                    for mi, m in enumerate(range(m_lo, m_hi + 1)):
                        j = c - 2 + m
                        pt = pt_tiles[(seq, j, h)]
                        b = c - j + 2
                        nc.tensor.matmul(
                            ctx, pt[:, b * 128:(b + 1) * 128],
                            v_sb[:, seq * CPS + j,
                                 h * (HD + 1):(h + 1) * (HD + 1)],
                            start=(mi == 0), stop=(mi == nm - 1))
                    rec = recp.tile([128, 1], F32, tag="rec")
                    nc.vector.reciprocal(rec[:], ctx[:, HD:HD + 1])
                    nc.vector.tensor_scalar_mul(cn[:, ci, h, :],
                                                ctx[:, 0:HD], rec[:])

            def transpose_pair(pi):
                # 2-chunk batched Ant transpose into feature-major ctxT;
                # dispatched well after the fixup so the SP queue never
                # blocks on it
                nc.sync.dma_start_transpose(
                    ctxT_sb[:, pi * 256:(pi + 1) * 256].rearrange(
                        "p (a b) -> p a b", a=2),
                    cn_state.pop(pi)[:].rearrange("p a b c -> p (a b c)"))

            def outproj_pair(pi):
                ob = outsb.tile([128, 2, E], BF16, tag="ob")
                for ci in range(2):
                    gc = pi * 2 + ci
                    for half in range(2):
                        op = ps512.tile([128, 512], F32, tag="ps512",
                                        name="op")
                        nc.tensor.matmul(
                            op[:], ctxT_sb[:, gc * 128:(gc + 1) * 128],
                            wo_sb[:, half * 512:(half + 1) * 512],
                            start=True, stop=True)
                        # drain copies alternate DVE/ACT so neither the exp
                        # chain nor the fixup chain eats the full cost
                        if (ci + half) % 2 == 0:
                            nc.vector.tensor_copy(
                                ob[:, ci, half * 512:(half + 1) * 512], op[:])
                        else:
                            nc.scalar.activation(
                                ob[:, ci, half * 512:(half + 1) * 512], op[:],
                                mybir.ActivationFunctionType.Copy)
                t0 = pi * 2 * 128
                nc.sync.dma_start(
                    out_d[t0:t0 + 256, :].rearrange("(c p) e -> p c e", p=128),
                    ob[:])

            # ---- fine-grained emission keyed on the key-tile index ----
            proj_chunks = 0
            next_slab = 0
            issued = 0

            def issue_ahead(depth):
                nonlocal issued
                while issued < min(next_slab + depth, len(slabs)):
                    issue_xT(issued)
                    issued += 1

            issue_ahead(2)
            pairs_done = 0
            op_done = 0

            def drain_outproj(limit_pairs):
                nonlocal op_done
                while op_done < limit_pairs:
                    outproj_pair(op_done)
                    op_done += 1

            for gj in range(NT):
                seq, j = divmod(gj, CPS)
                need = seq * CPS + min(j + 3, CPS - 1)
                pend_v = []
                while proj_chunks <= need:
                    issue_ahead(3)
                    si = next_slab
                    proj_qk(si)
                    if len(pend_v) >= 1:
                        proj_v(pend_v.pop(0))
                    pend_v.append(si)
                    proj_chunks += slabs[si][1] // 128
                    next_slab += 1
                for h in range(HPC):
                    scores_j(seq, j, h)
                for si in pend_v:
                    proj_v(si)
                if gj >= 2:
                    attention_chunk(gj - 2)
                if gj >= 5 and (gj - 5) % 2 == 0:
                    transpose_pair((gj - 5) // 2)
                    pairs_done += 1
                drain_outproj(pairs_done - 2)
            for gc in (NT - 2, NT - 1):
                attention_chunk(gc)
            while pairs_done < NT // 2:
                transpose_pair(pairs_done)
                pairs_done += 1
                drain_outproj(pairs_done - 1)
            drain_outproj(pairs_done)

    nc.compile()
    return nc


_NC_CACHE = None


def _get_program():
    global _NC_CACHE
    if _NC_CACHE is None:
        _NC_CACHE = _build_program()
    return _NC_CACHE


def make_core_inputs(x, Wqkv, bqkv, Wo):
    """Host-side shard prep: per-core reordered/transposed weight slices.
    bf16 is the on-device compute dtype; casting here (vs on-device) is
    numerically identical and saves a full f32 pass over x.  x is also
    transposed here so the device input stream is a plain DMA copy."""
    import ml_dtypes
    bf16 = ml_dtypes.bfloat16
    xbfT = np.ascontiguousarray(
        np.asarray(x).reshape(T, D).T).astype(bf16)
    in_maps = []
    for ci in range(NCORES):
        heads = [HPC * ci + i for i in range(HPC)]
        rows = []
        brows = []
        for comp in range(3):
            for h in heads:
                sl = slice(h * 3 * HD + comp * HD, h * 3 * HD + (comp + 1) * HD)
                rows.append(Wqkv[sl])
                brows.append(bqkv[sl])
        wq = np.ascontiguousarray(
            np.concatenate(rows, axis=0).T.astype(np.float32)).astype(bf16)
        bq = np.concatenate(brows).astype(np.float32)
        cols = np.concatenate([np.arange(h * HD, (h + 1) * HD) for h in heads])
        woT = np.ascontiguousarray(
            Wo[:, cols].T.astype(np.float32)).astype(bf16)
        in_maps.append({
            "xbfT": xbfT, "wqkvT": wq, "bqkv": bq, "woT": woT,
        })
    return in_maps


def _reference_numpy(x, padding_mask, Wqkv, bqkv, Wo, bo):
    """Exact fallback (only used if padding_mask is not all ones)."""
    NEG = -9e15
    Bx, Sx, Dx = x.shape
    Hh, hd, w = H, HD, W
    qkv = (x.reshape(-1, Dx) @ Wqkv.T + bqkv).reshape(Bx, Sx, Hh, 3, hd)
    q = np.transpose(qkv[..., 0, :], (0, 2, 1, 3))
    k = np.transpose(qkv[..., 1, :], (0, 2, 1, 3))
    v = np.transpose(qkv[..., 2, :], (0, 2, 1, 3))
    nb = Sx // w
    idx = (np.arange(nb) * w)[:, None] + np.arange(3 * w)[None, :]
    kp = np.pad(k, ((0, 0), (0, 0), (w, w), (0, 0)))
    vp = np.pad(v, ((0, 0), (0, 0), (w, w), (0, 0)))
    k_c = kp[:, :, idx, :]
    v_c = vp[:, :, idx, :]
    sc = np.einsum('bhnxd,bhnyd->bhnxy', q.reshape(Bx, Hh, nb, w, hd), k_c)
    x_i = np.arange(w)[:, None]
    j_i = x_i + np.arange(2 * w + 1)[None, :]
    band = sc[..., x_i, j_i]
    key_pos = np.arange(Sx).reshape(nb, w)[:, :, None] - w + np.arange(2 * w + 1)
    valid = (key_pos >= 0) & (key_pos < Sx)
    km = padding_mask[:, np.clip(key_pos, 0, Sx - 1)] != 0
    m = valid[None, None] & km[:, None]
    band = np.where(m, band, NEG)
    band = band / np.sqrt(hd)
    band = band - band.max(axis=-1, keepdims=True)
    e = np.exp(band)
    attn = e / e.sum(axis=-1, keepdims=True)
    attn = np.where(m, attn, 0.0)
    a3 = np.zeros_like(sc)
    a3[..., x_i, j_i] = attn
    ctx = np.einsum('bhnxy,bhnyd->bhnxd', a3, v_c).reshape(Bx, Hh, Sx, hd)
    out = np.transpose(ctx, (0, 2, 1, 3)).reshape(Bx, Sx, Hh * hd)
    return (out @ Wo.T + bo).astype(np.float32)


def kernel(x, padding_mask, Wqkv, bqkv, Wo, bo):
    x = np.asarray(x)
    padding_mask = np.asarray(padding_mask)
    Wqkv = np.asarray(Wqkv, dtype=np.float32)
    bqkv = np.asarray(bqkv, dtype=np.float32)
    Wo = np.asarray(Wo, dtype=np.float32)
    bo = np.asarray(bo, dtype=np.float32)
    if not np.all(padding_mask != 0):
        return _reference_numpy(x.astype(np.float32), padding_mask,
                                Wqkv, bqkv, Wo, bo)
    nc = _get_program()
    in_maps = make_core_inputs(x, Wqkv, bqkv, Wo)
    res = run_bass_kernel_spmd(nc, in_maps, core_ids=list(range(NCORES)))
    acc = np.zeros((T, E), np.float32)
    for ci in range(NCORES):
        acc += np.asarray(res.results[ci]["out_p"]).astype(np.float32)
    # the v bias is not applied on-device; attention rows sum to 1, so
    # ctx = P v0 / den + bv exactly, and its Wo image folds into bo here
    bv = bqkv.reshape(H, 3, HD)[:, 2, :].reshape(E)
    acc += (bo + bv @ Wo.T)[None, :]
    return acc.reshape(B, S, E)


# revision 22
# speedup vs baseline: 1.1931x; 1.1931x over previous
"""Banded (Longformer-style) multi-head attention on 8 TRN2 NeuronCores.

Sharding: 16 heads are split 2-per-core (tensor parallel on H); every
core sees all 8192 tokens.  Compute dtype is bf16 (f32 accumulate in
PSUM); inputs are pre-cast/pre-TRANSPOSED on the host, so the x input
stream is a plain strided DMA copy.

Per-core kernel (single NEFF, fine-grained software-pipelined emission
keyed on the 128-wide key tile index so proj / attention / out-proj
interleave at ~2.5us granularity on the PE):
  1. DMA feature-major xT slabs DRAM->SBUF; project to qT,kT
     (feature-major [d, T]) via w-stationary matmuls and to v TOKEN-major
     via x-stationary matmuls (same FLOPs, no v transpose).  v is stored
     ones-augmented so the P@V matmul also produces the softmax
     denominator.  The v bias is folded into the output bias on the host
     (exact: attention rows sum to 1).
  2. j-major banded attention: for each 128-wide key tile j, one
     scores^T matmul [key,y x query-cols] against the <=5 query chunks
     in its band (K=64), exp on ScalarE without max-subtraction
     (scores are O(+-30), exact in f32), band-corner masking via
     affine_select on the GpSimd engine, then per-query-chunk
     P^T@V_aug accumulation (K=128) and a 1/den fixup on VectorE.
  3. ctx 2-chunk groups transposed feature-major by the Ant DMA-transpose
     unit, dispatched 3+ chunks after their fixup so the SP queue never
     stalls holding the dispatch slot; partial output projection
     ctx_h @ Wo_h.T -> [8192, 1024] bf16 drained at chunk granularity.
The host sums the 8 partial outputs and adds the output bias (the
all-reduce step of tensor parallelism, done during the gather).
"""

import sys

sys.path.insert(0, "/opt/trn_rl_repo")

import numpy as np

import concourse.bass as bass
import concourse.mybir as mybir
import concourse.tile as tile
from concourse import bacc
from concourse.bass_utils import run_bass_kernel_spmd

F32 = mybir.dt.float32
BF16 = mybir.dt.bfloat16

B, S, D, E, H, HD = 2, 4096, 1024, 1024, 16, 64
W = 256                    # half window
T = B * S                  # 8192 flattened tokens
NCORES = 8
HPC = H // NCORES          # 2 heads per core
FQKV = 3 * HPC * HD        # 384 projected features per core
NT = T // 128              # 64 token chunks
CPS = S // 128             # 32 chunks per sequence
SLAB = 512                 # proj token slab
VROW = 2 * (HD + 1)        # 130: [v_h0(64) | 1 | v_h1(64) | 1]

# tuning knobs (swept offline via TimelineSim; see sweep.py)
CFG = dict(
    op_gran='chunk',    # 'chunk': 2 outproj mms/gj; 'pair': 4 every 2 gj
    ob_eng='dve',       # drain engine for outproj psum: dve|act|alt
    v_eng='act',        # drain engine for v psum
    qk_eng='dve',       # drain engine for q/k psum
    op_lag=4,           # outproj chunks held back behind transposes
    tp_lag=7,           # first transpose pair at gj = tp_lag
    ps_bufs=4,
    pt_bufs=12,
    look=2,             # proj lookahead in chunks beyond the band edge
    xtp_bufs=4,
    ctx_slots=2,        # PV accumulator depth in chunks (2 slots each)
    st_merged=False,    # single manually-rotated [128,2,640] score tile
    st_split=True,      # [128,512] main pool + packed corner bank
    stm_bufs=2,         # main score pool depth when st_split
    op_first=True,      # emit outproj drains before attention_chunk
    tail_eng='alt',     # tail drain engine: alt|dve|act
)


def _build_program(cfg=CFG):
    nc = bacc.Bacc(None, target_bir_lowering=False, debug=False)

    xT_d = nc.dram_tensor("xbfT", [D, T], BF16, kind="ExternalInput")
    wqkvT_d = nc.dram_tensor("wqkvT", [D, FQKV], BF16, kind="ExternalInput")
    bqkv_d = nc.dram_tensor("bqkv", [FQKV], F32, kind="ExternalInput")
    woT_d = nc.dram_tensor("woT", [HPC * HD, E], BF16, kind="ExternalInput")
    out_d = nc.dram_tensor("out_p", [T, E], BF16, kind="ExternalOutput")

    def cp_eng(which, i=0):
        name = cfg[which]
        if name == 'alt':
            name = ('dve', 'act')[i % 2]
        return nc.vector if name == 'dve' else nc.scalar

    def copy_with(eng, out, in_):
        if eng is nc.vector:
            nc.vector.tensor_copy(out, in_)
        else:
            nc.scalar.activation(out, in_,
                                 mybir.ActivationFunctionType.Copy)

    with tile.TileContext(nc) as tc:
        with (
            tc.tile_pool(name="const", bufs=1) as cpool,
            tc.tile_pool(name="big", bufs=1) as bigpool,
            tc.tile_pool(name="xtp", bufs=cfg['xtp_bufs']) as xtp,
            tc.tile_pool(name="cnp", bufs=4) as cnp,
            tc.tile_pool(name="recp", bufs=4) as recp,
            tc.tile_pool(name="ptp", bufs=cfg['pt_bufs']) as ptp,
            tc.tile_pool(name="outsb", bufs=2) as outsb,
            tc.tile_pool(name="ps512", bufs=cfg['ps_bufs'],
                         space="PSUM") as ps512,
            tc.tile_pool(name="spsum",
                         bufs=(1 if cfg['st_merged'] else
                               cfg['stm_bufs'] if cfg['st_split'] else 2),
                         space="PSUM") as spsum,
            tc.tile_pool(name="scp", bufs=1, space="PSUM") as scp,
            tc.tile_pool(name="cpsum", bufs=1, space="PSUM") as cpsum,
        ):
            # ---- constants; w_sb chunk 0 + xT slab 0 first so the first
            # proj matmul waits on <1.1MB of DMA ----
            w_sb = cpool.tile([128, 8, FQKV], BF16, tag="w_sb")
            nc.sync.dma_start(w_sb[:, 0, 0:128], wqkvT_d[0:128, 0:128])

            xT_tiles = {}
            slabs = ([(0, 128), (128, 128), (256, 256)] +
                     [(512 * k, 512) for k in range(1, T // 512)])

            def issue_xT(si):
                t0, wd = slabs[si]
                xT = xtp.tile([128, 8, SLAB], BF16, tag="xT")
                nc.sync.dma_start(
                    xT[:, :, 0:wd],
                    xT_d[:, t0:t0 + wd].rearrange("(c p) t -> p c t", p=128))
                xT_tiles[si] = xT

            issue_xT(0)
            nc.sync.dma_start(w_sb[:, 0, 128:FQKV], wqkvT_d[0:128, 128:FQKV])
            for c in range(1, 8):
                nc.sync.dma_start(
                    w_sb[:, c, :], wqkvT_d[c * 128:(c + 1) * 128, :])
            b_sb = cpool.tile([128, 3], F32, tag="b_sb")
            nc.sync.dma_start(b_sb[:], bqkv_d[:].rearrange("(a p) -> p a", p=128))
            issue_xT(1)
            issue_xT(2)
            wo_sb = cpool.tile([128, E], BF16, tag="wo_sb")
            nc.sync.dma_start(wo_sb[:], woT_d[:])
            issued = 3

            # ---- persistent activations ----
            q_sb = bigpool.tile([128, T], BF16, tag="q_sb")
            k_sb = bigpool.tile([128, T], BF16, tag="k_sb")
            v_sb = bigpool.tile([128, NT, VROW], BF16, tag="v_sb")
            ctxT_sb = bigpool.tile([128, T], BF16, tag="ctxT_sb")
            # ones columns of the augmented V (cols 64 and 129 of each chunk)
            nc.vector.memset(v_sb[:, :, HD::HD + 1], 1.0)

            # PV accumulators: 3 chunks x 2 heads packed in one PSUM bank
            ctx_ps = cpsum.tile([128, 2 * cfg['ctx_slots'], HD + 1], F32,
                                tag="ctx_ps", name="ctx_ps")
            # scores: one [128, 2, 640] f32 tile, manually rotated; matmul
            # pieces must not cross the 2KB PSUM bank boundaries, which sit
            # at col 512 for slot 0 and col 384 for slot 1
            st_ps = (spsum.tile([128, 2, 640], F32, tag="st_ps",
                                name="st_ps")
                     if cfg['st_merged'] else None)
            stc_ps = (scp.tile([128, 4, 128], F32, tag="stc_ps",
                               name="stc_ps")
                      if cfg['st_split'] else None)
            st_slot = [0]
            stc_slot = [0]

            def proj_qk(si):
                t0, ntok = slabs[si]
                xT = xT_tiles[si]
                for ft in range(2):
                    ps = ps512.tile([128, SLAB], F32, tag="ps512")
                    for c in range(8):
                        nc.tensor.matmul(
                            ps[:, 0:ntok], w_sb[:, c, ft * 128:(ft + 1) * 128],
                            xT[:, c, 0:ntok], start=(c == 0), stop=(c == 7))
                    dest = (q_sb, k_sb)[ft]
                    eng = cp_eng('qk_eng', ft)
                    if eng is nc.vector:
                        nc.vector.tensor_scalar_add(
                            dest[:, t0:t0 + ntok], ps[:, 0:ntok],
                            b_sb[:, ft:ft + 1])
                    else:
                        nc.scalar.activation(
                            dest[:, t0:t0 + ntok], ps[:, 0:ntok],
                            mybir.ActivationFunctionType.Copy,
                            bias=b_sb[:, ft:ft + 1])

            def proj_v(si):
                t0, ntok = slabs[si]
                nck = ntok // 128
                xT = xT_tiles.pop(si)
                # v: token-major [128 tokens, 128 feats] per chunk (no
                # transpose needed; v bias is folded into bo on the host)
                vps = ps512.tile([128, SLAB], F32, tag="ps512")
                for ck in range(nck):
                    for c in range(8):
                        nc.tensor.matmul(
                            vps[:, ck * 128:(ck + 1) * 128],
                            xT[:, c, ck * 128:(ck + 1) * 128],
                            w_sb[:, c, 2 * 128:3 * 128],
                            start=(c == 0), stop=(c == 7))
                for ck in range(nck):
                    gck = t0 // 128 + ck
                    copy_with(
                        cp_eng('v_eng', ck),
                        v_sb[:, gck, :].rearrange(
                            "p (h r) -> p h r", h=2)[:, :, 0:HD],
                        vps[:, ck * 128:(ck + 1) * 128].rearrange(
                            "p (h r) -> p h r", h=2))

            # j-major scoresT: st_j[y, b*128:(b+1)*128] = k_j^T q_{c}, where
            # c = j-2+b.  pt_j = exp(st_j/8) with band corners zeroed via
            # affine_select on GpSimd.
            pt_tiles = {}
            cn_state = {}

            def scores_j(seq, j, h):
                b_lo = max(0, 2 - j)
                b_hi = min(4, 2 + (CPS - 1) - j)
                gj = seq * CPS + j
                lo, hi = b_lo * 128, (b_hi + 1) * 128
                qcols = (seq * CPS + j - 2) * 128
                pt = ptp.tile([128, 640], BF16, tag="pt")
                if cfg['st_split']:
                    # main [lo, min(hi,512)) in a pooled bank tile; the b=4
                    # corner lives in a packed 4-slot bank of its own
                    mhi = min(hi, 512)
                    st = spsum.tile([128, 512], F32, tag="st", name="st")[:]
                    nc.tensor.matmul(
                        st[:, lo:mhi],
                        k_sb[h * HD:(h + 1) * HD, gj * 128:(gj + 1) * 128],
                        q_sb[h * HD:(h + 1) * HD, qcols + lo:qcols + mhi],
                        start=True, stop=True)
                    nc.scalar.activation(
                        pt[:, lo:mhi], st[:, lo:mhi],
                        mybir.ActivationFunctionType.Exp,
                        scale=float(1.0 / np.sqrt(HD)))
                    if hi > 512:
                        sc = stc_slot[0]
                        stc_slot[0] = (sc + 1) % 4
                        stc = stc_ps[:, sc, :]
                        nc.tensor.matmul(
                            stc,
                            k_sb[h * HD:(h + 1) * HD,
                                 gj * 128:(gj + 1) * 128],
                            q_sb[h * HD:(h + 1) * HD,
                                 qcols + 512:qcols + hi],
                            start=True, stop=True)
                        nc.scalar.activation(
                            pt[:, 512:hi], stc,
                            mybir.ActivationFunctionType.Exp,
                            scale=float(1.0 / np.sqrt(HD)))
                else:
                    if cfg['st_merged']:
                        sl = st_slot[0]
                        st_slot[0] ^= 1
                        st = st_ps[:, sl, :]
                        cuts = [c for c in ((512,) if sl == 0 else (384,))
                                if lo < c < hi]
                    else:
                        st = spsum.tile([128, 640], F32, tag="st",
                                        name="st")[:]
                        cuts = [c for c in (512,) if lo < c < hi]
                    edges = [lo] + cuts + [hi]
                    pieces = list(zip(edges[:-1], edges[1:]))
                    for (a, b) in pieces:
                        nc.tensor.matmul(
                            st[:, a:b],
                            k_sb[h * HD:(h + 1) * HD,
                                 gj * 128:(gj + 1) * 128],
                            q_sb[h * HD:(h + 1) * HD, qcols + a:qcols + b],
                            start=True, stop=True)
                    nc.scalar.activation(
                        pt[:, lo:hi], st[:, lo:hi],
                        mybir.ActivationFunctionType.Exp,
                        scale=float(1.0 / np.sqrt(HD)))
                if b_lo == 0:
                    # b=0 <-> chunk c=j-2, m=4: keep y <= t  (p <= f)
                    nc.gpsimd.affine_select(
                        out=pt[:, 0:128], in_=pt[:, 0:128],
                        compare_op=mybir.AluOpType.is_ge, fill=0.0, base=0,
                        pattern=[[1, 128]], channel_multiplier=-1)
                if b_hi == 4:
                    # b=4 <-> chunk c=j+2, m=0: keep y >= t  (p >= f)
                    nc.gpsimd.affine_select(
                        out=pt[:, 512:640], in_=pt[:, 512:640],
                        compare_op=mybir.AluOpType.is_ge, fill=0.0, base=0,
                        pattern=[[-1, 128]], channel_multiplier=1)
                pt_tiles[(seq, j, h)] = pt

            def attention_chunk(gc):
                seq, c = divmod(gc, CPS)
                qi, ci = divmod(gc, 2)
                m_lo = max(0, 2 - c)
                m_hi = min(4, CPS - 1 - c + 2)
                nm = m_hi - m_lo + 1
                if ci == 0:
                    cn = cnp.tile([128, 2, 2, HD], BF16, tag="cn", name="cn")
                    cn_state[qi] = cn
                cn = cn_state[qi]
                for h in range(HPC):
                    ctx = ctx_ps[:, (gc % cfg['ctx_slots']) * 2 + h, :]
                    for mi, m in enumerate(range(m_lo, m_hi + 1)):
                        j = c - 2 + m
                        pt = pt_tiles[(seq, j, h)]
                        b = c - j + 2
                        nc.tensor.matmul(
                            ctx, pt[:, b * 128:(b + 1) * 128],
                            v_sb[:, seq * CPS + j,
                                 h * (HD + 1):(h + 1) * (HD + 1)],
                            start=(mi == 0), stop=(mi == nm - 1))
                    rec = recp.tile([128, 1], F32, tag="rec")
                    nc.vector.reciprocal(rec[:], ctx[:, HD:HD + 1])
                    nc.vector.tensor_scalar_mul(cn[:, ci, h, :],
                                                ctx[:, 0:HD], rec[:])

            def transpose_pair(pi):
                # 2-chunk batched Ant transpose into feature-major ctxT;
                # dispatched well after the fixup so the SP queue never
                # blocks on it
                nc.sync.dma_start_transpose(
                    ctxT_sb[:, pi * 256:(pi + 1) * 256].rearrange(
                        "p (a b) -> p a b", a=2),
                    cn_state.pop(pi)[:].rearrange("p a b c -> p (a b c)"))

            ob_state = {}

            def outproj_chunk(gc, tail=False):
                pi, ci = divmod(gc, 2)
                if ci == 0:
                    ob = outsb.tile([128, 2, E], BF16, tag="ob", name="ob")
                    ob_state[pi] = ob
                ob = ob_state[pi]
                for half in range(2):
                    op = ps512.tile([128, 512], F32, tag="ps512", name="op")
                    nc.tensor.matmul(
                        op[:], ctxT_sb[:, gc * 128:(gc + 1) * 128],
                        wo_sb[:, half * 512:(half + 1) * 512],
                        start=True, stop=True)
                    if tail:
                        tn = cfg['tail_eng']
                        eng = ((nc.vector, nc.scalar)[(gc + half) % 2]
                               if tn == 'alt' else
                               nc.vector if tn == 'dve' else nc.scalar)
                    else:
                        eng = cp_eng('ob_eng', gc + half)
                    copy_with(eng,
                              ob[:, ci, half * 512:(half + 1) * 512], op[:])
                if ci == 1:
                    t0 = pi * 2 * 128
                    nc.sync.dma_start(
                        out_d[t0:t0 + 256, :].rearrange(
                            "(c p) e -> p c e", p=128),
                        ob_state.pop(pi)[:])

            # ---- fine-grained emission keyed on the key-tile index ----
            proj_chunks = 0
            next_slab = 0

            def issue_ahead(depth):
                nonlocal issued
                while issued < min(next_slab + depth, len(slabs)):
                    issue_xT(issued)
                    issued += 1

            pairs_done = 0
            op_done = 0

            def drain_outproj(limit_chunks, maxn=100, tail=False):
                nonlocal op_done
                while op_done < limit_chunks and maxn > 0:
                    outproj_chunk(op_done, tail=tail)
                    op_done += 1
                    maxn -= 1

            npair = 1 if cfg['op_gran'] == 'chunk' else 2
            for gj in range(NT):
                seq, j = divmod(gj, CPS)
                need = seq * CPS + min(j + cfg['look'], CPS - 1)
                pend_v = []
                while proj_chunks <= need:
                    issue_ahead(3)
                    si = next_slab
                    proj_qk(si)
                    if pend_v:
                        proj_v(pend_v.pop(0))
                    pend_v.append(si)
                    proj_chunks += slabs[si][1] // 128
                    next_slab += 1
                for h in range(HPC):
                    scores_j(seq, j, h)
                if cfg['op_first']:
                    drain_outproj(pairs_done * 2 - cfg['op_lag'], npair)
                for si in pend_v:
                    proj_v(si)
                if gj >= 2:
                    attention_chunk(gj - 2)
                while (pairs_done < (gj - cfg['tp_lag']) // 2 + 1
                       and pairs_done * 2 + 1 <= gj - 2):
                    transpose_pair(pairs_done)
                    pairs_done += 1
                if not cfg['op_first']:
                    drain_outproj(pairs_done * 2 - cfg['op_lag'], npair)
            for gc in (NT - 2, NT - 1):
                attention_chunk(gc)
            while pairs_done < NT // 2:
                transpose_pair(pairs_done)
                pairs_done += 1
                drain_outproj(pairs_done * 2 - 2, tail=True)
            drain_outproj(NT, tail=True)

    nc.compile()
    return nc


_NC_CACHE = None


def _get_program():
    global _NC_CACHE
    if _NC_CACHE is None:
        _NC_CACHE = _build_program()
    return _NC_CACHE


def make_core_inputs(x, Wqkv, bqkv, Wo):
    """Host-side shard prep: per-core reordered/transposed weight slices.
    bf16 is the on-device compute dtype; casting here (vs on-device) is
    numerically identical and saves a full f32 pass over x.  x is also
    transposed here so the device input stream is a plain DMA copy."""
    import ml_dtypes
    bf16 = ml_dtypes.bfloat16
    xbfT = np.ascontiguousarray(
        np.asarray(x).reshape(T, D).T).astype(bf16)
    in_maps = []
    for ci in range(NCORES):
        heads = [HPC * ci + i for i in range(HPC)]
        rows = []
        brows = []
        for comp in range(3):
            for h in heads:
                sl = slice(h * 3 * HD + comp * HD, h * 3 * HD + (comp + 1) * HD)
                rows.append(Wqkv[sl])
                brows.append(bqkv[sl])
        wq = np.ascontiguousarray(
            np.concatenate(rows, axis=0).T.astype(np.float32)).astype(bf16)
        bq = np.concatenate(brows).astype(np.float32)
        cols = np.concatenate([np.arange(h * HD, (h + 1) * HD) for h in heads])
        woT = np.ascontiguousarray(
            Wo[:, cols].T.astype(np.float32)).astype(bf16)
        in_maps.append({
            "xbfT": xbfT, "wqkvT": wq, "bqkv": bq, "woT": woT,
        })
    return in_maps


def _reference_numpy(x, padding_mask, Wqkv, bqkv, Wo, bo):
    """Exact fallback (only used if padding_mask is not all ones)."""
    NEG = -9e15
    Bx, Sx, Dx = x.shape
    Hh, hd, w = H, HD, W
    qkv = (x.reshape(-1, Dx) @ Wqkv.T + bqkv).reshape(Bx, Sx, Hh, 3, hd)
    q = np.transpose(qkv[..., 0, :], (0, 2, 1, 3))
    k = np.transpose(qkv[..., 1, :], (0, 2, 1, 3))
    v = np.transpose(qkv[..., 2, :], (0, 2, 1, 3))
    nb = Sx // w
    idx = (np.arange(nb) * w)[:, None] + np.arange(3 * w)[None, :]
    kp = np.pad(k, ((0, 0), (0, 0), (w, w), (0, 0)))
    vp = np.pad(v, ((0, 0), (0, 0), (w, w), (0, 0)))
    k_c = kp[:, :, idx, :]
    v_c = vp[:, :, idx, :]
    sc = np.einsum('bhnxd,bhnyd->bhnxy', q.reshape(Bx, Hh, nb, w, hd), k_c)
    x_i = np.arange(w)[:, None]
    j_i = x_i + np.arange(2 * w + 1)[None, :]
    band = sc[..., x_i, j_i]
    key_pos = np.arange(Sx).reshape(nb, w)[:, :, None] - w + np.arange(2 * w + 1)
    valid = (key_pos >= 0) & (key_pos < Sx)
    km = padding_mask[:, np.clip(key_pos, 0, Sx - 1)] != 0
    m = valid[None, None] & km[:, None]
    band = np.where(m, band, NEG)
    band = band / np.sqrt(hd)
    band = band - band.max(axis=-1, keepdims=True)
    e = np.exp(band)
    attn = e / e.sum(axis=-1, keepdims=True)
    attn = np.where(m, attn, 0.0)
    a3 = np.zeros_like(sc)
    a3[..., x_i, j_i] = attn
    ctx = np.einsum('bhnxy,bhnyd->bhnxd', a3, v_c).reshape(Bx, Hh, Sx, hd)
    out = np.transpose(ctx, (0, 2, 1, 3)).reshape(Bx, Sx, Hh * hd)
    return (out @ Wo.T + bo).astype(np.float32)


def kernel(x, padding_mask, Wqkv, bqkv, Wo, bo):
    x = np.asarray(x)
    padding_mask = np.asarray(padding_mask)
    Wqkv = np.asarray(Wqkv, dtype=np.float32)
    bqkv = np.asarray(bqkv, dtype=np.float32)
    Wo = np.asarray(Wo, dtype=np.float32)
    bo = np.asarray(bo, dtype=np.float32)
    if not np.all(padding_mask != 0):
        return _reference_numpy(x.astype(np.float32), padding_mask,
                                Wqkv, bqkv, Wo, bo)
    nc = _get_program()
    in_maps = make_core_inputs(x, Wqkv, bqkv, Wo)
    res = run_bass_kernel_spmd(nc, in_maps, core_ids=list(range(NCORES)))
    acc = np.zeros((T, E), np.float32)
    for ci in range(NCORES):
        acc += np.asarray(res.results[ci]["out_p"]).astype(np.float32)
    # the v bias is not applied on-device; attention rows sum to 1, so
    # ctx = P v0 / den + bv exactly, and its Wo image folds into bo here
    bv = bqkv.reshape(H, 3, HD)[:, 2, :].reshape(E)
    acc += (bo + bv @ Wo.T)[None, :]
    return acc.reshape(B, S, E)


# revision 29
# speedup vs baseline: 1.1965x; 1.0028x over previous
"""Banded (Longformer-style) multi-head attention on 8 TRN2 NeuronCores.

Sharding: 16 heads are split 2-per-core (tensor parallel on H); every
core sees all 8192 tokens.  Compute dtype is bf16 (f32 accumulate in
PSUM); inputs are pre-cast/pre-TRANSPOSED on the host, so the x input
stream is a plain strided DMA copy.

Per-core kernel (single NEFF, fine-grained software-pipelined emission
keyed on the 128-wide key tile index so proj / attention / out-proj
interleave at ~2.5us granularity on the PE):
  1. DMA feature-major xT slabs DRAM->SBUF; project to qT,kT
     (feature-major [d, T]) via w-stationary matmuls and to v TOKEN-major
     via x-stationary matmuls (same FLOPs, no v transpose).  v is stored
     ones-augmented so the P@V matmul also produces the softmax
     denominator.  The v bias is folded into the output bias on the host
     (exact: attention rows sum to 1).
  2. j-major banded attention: for each 128-wide key tile j, one
     scores^T matmul [key,y x query-cols] against the <=5 query chunks
     in its band (K=64), exp on ScalarE without max-subtraction
     (scores are O(+-30), exact in f32), band-corner masking via
     affine_select on the GpSimd engine, then per-query-chunk
     P^T@V_aug accumulation (K=128) and a 1/den fixup on VectorE.
  3. ctx 2-chunk groups transposed feature-major by the Ant DMA-transpose
     unit, dispatched 3+ chunks after their fixup so the SP queue never
     stalls holding the dispatch slot; partial output projection
     ctx_h @ Wo_h.T -> [8192, 1024] bf16 drained at chunk granularity.
The host sums the 8 partial outputs and adds the output bias (the
all-reduce step of tensor parallelism, done during the gather).
"""

import sys

sys.path.insert(0, "/opt/trn_rl_repo")

import numpy as np

import concourse.bass as bass
import concourse.mybir as mybir
import concourse.tile as tile
from concourse import bacc
from concourse.bass_utils import run_bass_kernel_spmd

F32 = mybir.dt.float32
BF16 = mybir.dt.bfloat16

B, S, D, E, H, HD = 2, 4096, 1024, 1024, 16, 64
W = 256                    # half window
T = B * S                  # 8192 flattened tokens
NCORES = 8
HPC = H // NCORES          # 2 heads per core
FQKV = 3 * HPC * HD        # 384 projected features per core
NT = T // 128              # 64 token chunks
CPS = S // 128             # 32 chunks per sequence
SLAB = 512                 # proj token slab
VROW = 2 * (HD + 1)        # 130: [v_h0(64) | 1 | v_h1(64) | 1]

# tuning knobs (swept offline via TimelineSim; see sweep.py)
CFG = dict(
    op_gran='chunk',    # 'chunk': 2 outproj mms/gj; 'pair': 4 every 2 gj
    ob_eng='dve',       # drain engine for outproj psum: dve|act|alt
    v_eng='act',        # drain engine for v psum (dve|act|alt)
    qk_eng='dve',       # drain engine for q/k psum
    op_lag=4,           # outproj chunks held back behind transposes
    tp_lag=7,           # first transpose pair at gj = tp_lag
    ps_bufs=4,
    pt_bufs=16,
    look=2,             # proj lookahead in chunks beyond the band edge
    xtp_bufs=4,
    ctx_slots=2,        # PV accumulator depth in chunks (2 slots each)
    st_merged=False,    # single manually-rotated [128,2,640] score tile
    st_split=True,      # [128,512] main pool + packed corner bank
    stm_bufs=2,         # main score pool depth when st_split
    op_first=True,      # emit outproj drains before attention_chunk
    tail_eng='alt',     # tail drain engine: alt|dve|act
    first_slab='whole',  # 'split': 128/128/256 warmup slabs; 'whole': one 512
    tail_order='mixed',  # 'tp_first': dispatch all tail transposes first
    op_split=False,      # emit 1 op chunk before att and 1 after
    ob_dma_eng='sp',     # queue for output writes: sp|act|act_tail|pool|pool_tail
    xt_eng='sp',         # queue for xT loads: sp|pool
    proj_pieces=False,   # spread each slab as ft0/ft1/v pieces across gjs
    spread=2,            # how many gjs before its deadline a piece may run
    fix_late=False,      # emit rec/fixup after both heads' PV groups
    ctx_split=False,     # per-head ctx PSUM tiles (needs ps_bufs<=3)
)


def _build_program(cfg=CFG):
    nc = bacc.Bacc(None, target_bir_lowering=False, debug=False)

    xT_d = nc.dram_tensor("xbfT", [D, T], BF16, kind="ExternalInput")
    wqkvT_d = nc.dram_tensor("wqkvT", [D, FQKV], BF16, kind="ExternalInput")
    bqkv_d = nc.dram_tensor("bqkv", [FQKV], F32, kind="ExternalInput")
    woT_d = nc.dram_tensor("woT", [HPC * HD, E], BF16, kind="ExternalInput")
    out_d = nc.dram_tensor("out_p", [T, E], BF16, kind="ExternalOutput")

    def cp_eng(which, i=0):
        name = cfg[which]
        if name == 'alt':
            name = ('dve', 'act')[i % 2]
        return nc.vector if name == 'dve' else nc.scalar

    def copy_with(eng, out, in_):
        if eng is nc.vector:
            nc.vector.tensor_copy(out, in_)
        else:
            nc.scalar.activation(out, in_,
                                 mybir.ActivationFunctionType.Copy)

    with tile.TileContext(nc) as tc:
        with (
            tc.tile_pool(name="const", bufs=1) as cpool,
            tc.tile_pool(name="big", bufs=1) as bigpool,
            tc.tile_pool(name="xtp", bufs=cfg['xtp_bufs']) as xtp,
            tc.tile_pool(name="cnp", bufs=cfg.get('cnp_bufs', 4)) as cnp,
            tc.tile_pool(name="recp", bufs=cfg.get('recp_bufs', 4)) as recp,
            tc.tile_pool(name="ptp", bufs=cfg['pt_bufs']) as ptp,
            tc.tile_pool(name="outsb", bufs=2) as outsb,
            tc.tile_pool(name="ps512", bufs=cfg['ps_bufs'],
                         space="PSUM") as ps512,
            tc.tile_pool(name="spsum",
                         bufs=(1 if cfg['st_merged'] else
                               cfg['stm_bufs'] if cfg['st_split'] else 2),
                         space="PSUM") as spsum,
            tc.tile_pool(name="scp", bufs=1, space="PSUM") as scp,
            tc.tile_pool(name="cpsum", bufs=1, space="PSUM") as cpsum,
        ):
            # ---- constants; w_sb chunk 0 + xT slab 0 first so the first
            # proj matmul waits on <1.1MB of DMA ----
            w_sb = cpool.tile([128, 8, FQKV], BF16, tag="w_sb")
            nc.sync.dma_start(w_sb[:, 0, 0:128], wqkvT_d[0:128, 0:128])

            xT_tiles = {}
            slabs = (([(0, 128), (128, 128), (256, 256)]
                      if cfg['first_slab'] == 'split' else [(0, 512)]) +
                     [(512 * k, 512) for k in range(1, T // 512)])

            def issue_xT(si):
                t0, wd = slabs[si]
                xT = xtp.tile([128, 8, SLAB], BF16, tag="xT")
                dma = (nc.gpsimd.dma_start if cfg['xt_eng'] == 'pool'
                       else nc.sync.dma_start)
                dma(
                    xT[:, :, 0:wd],
                    xT_d[:, t0:t0 + wd].rearrange("(c p) t -> p c t", p=128))
                xT_tiles[si] = xT

            issue_xT(0)
            nc.sync.dma_start(w_sb[:, 0, 128:FQKV], wqkvT_d[0:128, 128:FQKV])
            for c in range(1, 8):
                nc.sync.dma_start(
                    w_sb[:, c, :], wqkvT_d[c * 128:(c + 1) * 128, :])
            b_sb = cpool.tile([128, 3], F32, tag="b_sb")
            nc.sync.dma_start(b_sb[:], bqkv_d[:].rearrange("(a p) -> p a", p=128))
            if cfg['first_slab'] == 'split':
                issue_xT(1)
                issue_xT(2)
                issued = 3
            else:
                issue_xT(1)
                issued = 2
            wo_sb = cpool.tile([128, E], BF16, tag="wo_sb")
            nc.sync.dma_start(wo_sb[:], woT_d[:])

            # ---- persistent activations ----
            q_sb = bigpool.tile([128, T], BF16, tag="q_sb")
            k_sb = bigpool.tile([128, T], BF16, tag="k_sb")
            v_sb = bigpool.tile([128, NT, VROW], BF16, tag="v_sb")
            ctxT_sb = bigpool.tile([128, T], BF16, tag="ctxT_sb")
            # ones columns of the augmented V (cols 64 and 129 of each chunk)
            nc.vector.memset(v_sb[:, :, HD::HD + 1], 1.0)

            # PV accumulators packed into PSUM banks
            if cfg['ctx_split']:
                ctx_a = cpsum.tile([128, cfg['ctx_slots'], HD + 1], F32,
                                   tag="ctx_a", name="ctx_a")
                ctx_b = cpsum.tile([128, cfg['ctx_slots'], HD + 1], F32,
                                   tag="ctx_b", name="ctx_b")
            else:
                ctx_ps = cpsum.tile([128, 2 * cfg['ctx_slots'], HD + 1], F32,
                                    tag="ctx_ps", name="ctx_ps")
            # scores: one [128, 2, 640] f32 tile, manually rotated; matmul
            # pieces must not cross the 2KB PSUM bank boundaries, which sit
            # at col 512 for slot 0 and col 384 for slot 1
            st_ps = (spsum.tile([128, 2, 640], F32, tag="st_ps",
                                name="st_ps")
                     if cfg['st_merged'] else None)
            stc_ps = (scp.tile([128, 4, 128], F32, tag="stc_ps",
                               name="stc_ps")
                      if cfg['st_split'] else None)
            st_slot = [0]
            stc_slot = [0]

            def proj_ft(si, ft):
                t0, ntok = slabs[si]
                xT = xT_tiles[si]
                ps = ps512.tile([128, SLAB], F32, tag="ps512")
                for c in range(8):
                    nc.tensor.matmul(
                        ps[:, 0:ntok], w_sb[:, c, ft * 128:(ft + 1) * 128],
                        xT[:, c, 0:ntok], start=(c == 0), stop=(c == 7))
                dest = (q_sb, k_sb)[ft]
                eng = cp_eng('qk_eng', ft)
                if eng is nc.vector:
                    nc.vector.tensor_scalar_add(
                        dest[:, t0:t0 + ntok], ps[:, 0:ntok],
                        b_sb[:, ft:ft + 1])
                else:
                    nc.scalar.activation(
                        dest[:, t0:t0 + ntok], ps[:, 0:ntok],
                        mybir.ActivationFunctionType.Copy,
                        bias=b_sb[:, ft:ft + 1])

            def proj_qk(si):
                proj_ft(si, 0)
                proj_ft(si, 1)

            def proj_v(si):
                t0, ntok = slabs[si]
                nck = ntok // 128
                xT = xT_tiles.pop(si)
                # v: token-major [128 tokens, 128 feats] per chunk (no
                # transpose needed; v bias is folded into bo on the host)
                vps = ps512.tile([128, SLAB], F32, tag="ps512")
                for ck in range(nck):
                    for c in range(8):
                        nc.tensor.matmul(
                            vps[:, ck * 128:(ck + 1) * 128],
                            xT[:, c, ck * 128:(ck + 1) * 128],
                            w_sb[:, c, 2 * 128:3 * 128],
                            start=(c == 0), stop=(c == 7))
                for ck in range(nck):
                    gck = t0 // 128 + ck
                    copy_with(
                        cp_eng('v_eng', ck),
                        v_sb[:, gck, :].rearrange(
                            "p (h r) -> p h r", h=2)[:, :, 0:HD],
                        vps[:, ck * 128:(ck + 1) * 128].rearrange(
                            "p (h r) -> p h r", h=2))

            # j-major scoresT: st_j[y, b*128:(b+1)*128] = k_j^T q_{c}, where
            # c = j-2+b.  pt_j = exp(st_j/8) with band corners zeroed via
            # affine_select on GpSimd.
            pt_tiles = {}
            cn_state = {}

            def scores_j(seq, j, h):
                b_lo = max(0, 2 - j)
                b_hi = min(4, 2 + (CPS - 1) - j)
                gj = seq * CPS + j
                lo, hi = b_lo * 128, (b_hi + 1) * 128
                qcols = (seq * CPS + j - 2) * 128
                pt = ptp.tile([128, 640], BF16, tag="pt")
                if cfg['st_split']:
                    # main [lo, min(hi,512)) in a pooled bank tile; the b=4
                    # corner lives in a packed 4-slot bank of its own
                    mhi = min(hi, 512)
                    st = spsum.tile([128, 512], F32, tag="st", name="st")[:]
                    nc.tensor.matmul(
                        st[:, lo:mhi],
                        k_sb[h * HD:(h + 1) * HD, gj * 128:(gj + 1) * 128],
                        q_sb[h * HD:(h + 1) * HD, qcols + lo:qcols + mhi],
                        start=True, stop=True)
                    nc.scalar.activation(
                        pt[:, lo:mhi], st[:, lo:mhi],
                        mybir.ActivationFunctionType.Exp,
                        scale=float(1.0 / np.sqrt(HD)))
                    if hi > 512:
                        sc = stc_slot[0]
                        stc_slot[0] = (sc + 1) % 4
                        stc = stc_ps[:, sc, :]
                        nc.tensor.matmul(
                            stc,
                            k_sb[h * HD:(h + 1) * HD,
                                 gj * 128:(gj + 1) * 128],
                            q_sb[h * HD:(h + 1) * HD,
                                 qcols + 512:qcols + hi],
                            start=True, stop=True)
                        nc.scalar.activation(
                            pt[:, 512:hi], stc,
                            mybir.ActivationFunctionType.Exp,
                            scale=float(1.0 / np.sqrt(HD)))
                else:
                    if cfg['st_merged']:
                        sl = st_slot[0]
                        st_slot[0] ^= 1
                        st = st_ps[:, sl, :]
                        cuts = [c for c in ((512,) if sl == 0 else (384,))
                                if lo < c < hi]
                    else:
                        st = spsum.tile([128, 640], F32, tag="st",
                                        name="st")[:]
                        cuts = [c for c in (512,) if lo < c < hi]
                    edges = [lo] + cuts + [hi]
                    pieces = list(zip(edges[:-1], edges[1:]))
                    for (a, b) in pieces:
                        nc.tensor.matmul(
                            st[:, a:b],
                            k_sb[h * HD:(h + 1) * HD,
                                 gj * 128:(gj + 1) * 128],
                            q_sb[h * HD:(h + 1) * HD, qcols + a:qcols + b],
                            start=True, stop=True)
                    nc.scalar.activation(
                        pt[:, lo:hi], st[:, lo:hi],
                        mybir.ActivationFunctionType.Exp,
                        scale=float(1.0 / np.sqrt(HD)))
                if b_lo == 0:
                    # b=0 <-> chunk c=j-2, m=4: keep y <= t  (p <= f)
                    nc.gpsimd.affine_select(
                        out=pt[:, 0:128], in_=pt[:, 0:128],
                        compare_op=mybir.AluOpType.is_ge, fill=0.0, base=0,
                        pattern=[[1, 128]], channel_multiplier=-1)
                if b_hi == 4:
                    # b=4 <-> chunk c=j+2, m=0: keep y >= t  (p >= f)
                    nc.gpsimd.affine_select(
                        out=pt[:, 512:640], in_=pt[:, 512:640],
                        compare_op=mybir.AluOpType.is_ge, fill=0.0, base=0,
                        pattern=[[-1, 128]], channel_multiplier=1)
                pt_tiles[(seq, j, h)] = pt

            def attention_chunk(gc):
                seq, c = divmod(gc, CPS)
                qi, ci = divmod(gc, 2)
                m_lo = max(0, 2 - c)
                m_hi = min(4, CPS - 1 - c + 2)
                nm = m_hi - m_lo + 1
                if ci == 0:
                    cn = cnp.tile([128, 2, 2, HD], BF16, tag="cn", name="cn")
                    cn_state[qi] = cn
                cn = cn_state[qi]
                ctxs = []
                for h in range(HPC):
                    if cfg['ctx_split']:
                        tile_h = (ctx_a, ctx_b)[h]
                        ctx = tile_h[:, gc % cfg['ctx_slots'], :]
                    else:
                        ctx = ctx_ps[:, (gc % cfg['ctx_slots']) * 2 + h, :]
                    ctxs.append(ctx)
                    for mi, m in enumerate(range(m_lo, m_hi + 1)):
                        j = c - 2 + m
                        pt = pt_tiles[(seq, j, h)]
                        b = c - j + 2
                        nc.tensor.matmul(
                            ctx, pt[:, b * 128:(b + 1) * 128],
                            v_sb[:, seq * CPS + j,
                                 h * (HD + 1):(h + 1) * (HD + 1)],
                            start=(mi == 0), stop=(mi == nm - 1))
                    if not cfg['fix_late']:
                        rec = recp.tile([128, 1], F32, tag="rec")
                        nc.vector.reciprocal(rec[:], ctx[:, HD:HD + 1])
                        nc.vector.tensor_scalar_mul(cn[:, ci, h, :],
                                                    ctx[:, 0:HD], rec[:])
                if cfg['fix_late']:
                    for h in range(HPC):
                        ctx = ctxs[h]
                        rec = recp.tile([128, 1], F32, tag="rec")
                        nc.vector.reciprocal(rec[:], ctx[:, HD:HD + 1])
                        nc.vector.tensor_scalar_mul(cn[:, ci, h, :],
                                                    ctx[:, 0:HD], rec[:])

            def transpose_pair(pi):
                # 2-chunk batched Ant transpose into feature-major ctxT;
                # dispatched well after the fixup so the SP queue never
                # blocks on it
                nc.sync.dma_start_transpose(
                    ctxT_sb[:, pi * 256:(pi + 1) * 256].rearrange(
                        "p (a b) -> p a b", a=2),
                    cn_state.pop(pi)[:].rearrange("p a b c -> p (a b c)"))

            ob_state = {}

            def outproj_chunk(gc, tail=False):
                pi, ci = divmod(gc, 2)
                if ci == 0:
                    ob = outsb.tile([128, 2, E], BF16, tag="ob", name="ob")
                    ob_state[pi] = ob
                ob = ob_state[pi]
                for half in range(2):
                    op = ps512.tile([128, 512], F32, tag="ps512", name="op")
                    nc.tensor.matmul(
                        op[:], ctxT_sb[:, gc * 128:(gc + 1) * 128],
                        wo_sb[:, half * 512:(half + 1) * 512],
                        start=True, stop=True)
                    if tail:
                        tn = cfg['tail_eng']
                        eng = ((nc.vector, nc.scalar)[(gc + half) % 2]
                               if tn == 'alt' else
                               nc.vector if tn == 'dve' else nc.scalar)
                    else:
                        eng = cp_eng('ob_eng', gc + half)
                    copy_with(eng,
                              ob[:, ci, half * 512:(half + 1) * 512], op[:])
                if ci == 1:
                    t0 = pi * 2 * 128
                    mode = cfg['ob_dma_eng']
                    tail_pi = pi >= NT // 2 - 3
                    if mode == 'act' or (mode == 'act_tail' and tail_pi):
                        dma = nc.scalar.dma_start
                    elif mode == 'pool' or (mode == 'pool_tail' and tail_pi):
                        dma = nc.gpsimd.dma_start
                    else:
                        dma = nc.sync.dma_start
                    dma(out_d[t0:t0 + 256, :].rearrange(
                            "(c p) e -> p c e", p=128),
                        ob_state.pop(pi)[:])

            # ---- fine-grained emission keyed on the key-tile index ----
            proj_chunks = 0
            next_slab = 0

            def issue_ahead(depth):
                nonlocal issued
                while issued < min(next_slab + depth, len(slabs)):
                    issue_xT(issued)
                    issued += 1

            pairs_done = 0
            op_done = 0

            def drain_outproj(limit_chunks, maxn=100, tail=False):
                nonlocal op_done
                while op_done < limit_chunks and maxn > 0:
                    outproj_chunk(op_done, tail=tail)
                    op_done += 1
                    maxn -= 1

            npair = 1 if cfg['op_gran'] == 'chunk' else 2

            def proj_piece(si, kind):
                if kind == 'v':
                    proj_v(si)
                else:
                    proj_ft(si, 0 if kind == 'ft0' else 1)

            def dl_qk(si):
                c0 = slabs[si][0] // 128
                return max(c0 - 2, (c0 // CPS) * CPS)

            def dl_v(si):
                c0 = slabs[si][0] // 128
                return max(c0 - 2, (c0 // CPS) * CPS) + 2

            from collections import deque
            pieces = deque()

            def piece_due(item, gj):
                si, kind = item
                return (dl_v(si) if kind == 'v' else dl_qk(si)) <= gj

            for gj in range(NT):
                seq, j = divmod(gj, CPS)
                if cfg['proj_pieces']:
                    while (next_slab < len(slabs) and
                           gj >= dl_qk(next_slab) - cfg['spread']):
                        issue_ahead(2)
                        for kind in ('ft0', 'ft1', 'v'):
                            pieces.append((next_slab, kind))
                        next_slab += 1
                    while pieces and piece_due(pieces[0], gj):
                        proj_piece(*pieces.popleft())
                else:
                    need = seq * CPS + min(j + cfg['look'], CPS - 1)
                    pend_v = []
                    while proj_chunks <= need:
                        issue_ahead(3)
                        si = next_slab
                        proj_qk(si)
                        if pend_v:
                            proj_v(pend_v.pop(0))
                        pend_v.append(si)
                        proj_chunks += slabs[si][1] // 128
                        next_slab += 1
                for h in range(HPC):
                    scores_j(seq, j, h)
                if cfg['proj_pieces'] and pieces:
                    proj_piece(*pieces.popleft())
                if cfg['op_first']:
                    drain_outproj(pairs_done * 2 - cfg['op_lag'],
                                  1 if cfg['op_split'] else npair)
                if not cfg['proj_pieces']:
                    for si in pend_v:
                        proj_v(si)
                if gj >= 2:
                    attention_chunk(gj - 2)
                while (pairs_done < (gj - cfg['tp_lag']) // 2 + 1
                       and pairs_done * 2 + 1 <= gj - 2):
                    transpose_pair(pairs_done)
                    pairs_done += 1
                if not cfg['op_first'] or cfg['op_split']:
                    drain_outproj(pairs_done * 2 - cfg['op_lag'],
                                  1 if cfg['op_split'] else npair)
            for gc in (NT - 2, NT - 1):
                attention_chunk(gc)
            if cfg['tail_order'] == 'tp_first':
                while pairs_done < NT // 2:
                    transpose_pair(pairs_done)
                    pairs_done += 1
                drain_outproj(NT, tail=True)
            else:
                while pairs_done < NT // 2:
                    transpose_pair(pairs_done)
                    pairs_done += 1
                    drain_outproj(pairs_done * 2 - 2, tail=True)
                drain_outproj(NT, tail=True)

    nc.compile()
    return nc


_NC_CACHE = None


def _get_program():
    global _NC_CACHE
    if _NC_CACHE is None:
        _NC_CACHE = _build_program()
    return _NC_CACHE


def make_core_inputs(x, Wqkv, bqkv, Wo):
    """Host-side shard prep: per-core reordered/transposed weight slices.
    bf16 is the on-device compute dtype; casting here (vs on-device) is
    numerically identical and saves a full f32 pass over x.  x is also
    transposed here so the device input stream is a plain DMA copy."""
    import ml_dtypes
    bf16 = ml_dtypes.bfloat16
    xbfT = np.ascontiguousarray(
        np.asarray(x).reshape(T, D).T).astype(bf16)
    in_maps = []
    for ci in range(NCORES):
        heads = [HPC * ci + i for i in range(HPC)]
        rows = []
        brows = []
        for comp in range(3):
            for h in heads:
                sl = slice(h * 3 * HD + comp * HD, h * 3 * HD + (comp + 1) * HD)
                rows.append(Wqkv[sl])
                brows.append(bqkv[sl])
        wq = np.ascontiguousarray(
            np.concatenate(rows, axis=0).T.astype(np.float32)).astype(bf16)
        bq = np.concatenate(brows).astype(np.float32)
        cols = np.concatenate([np.arange(h * HD, (h + 1) * HD) for h in heads])
        woT = np.ascontiguousarray(
            Wo[:, cols].T.astype(np.float32)).astype(bf16)
        in_maps.append({
            "xbfT": xbfT, "wqkvT": wq, "bqkv": bq, "woT": woT,
        })
    return in_maps


def _reference_numpy(x, padding_mask, Wqkv, bqkv, Wo, bo):
    """Exact fallback (only used if padding_mask is not all ones)."""
    NEG = -9e15
    Bx, Sx, Dx = x.shape
    Hh, hd, w = H, HD, W
    qkv = (x.reshape(-1, Dx) @ Wqkv.T + bqkv).reshape(Bx, Sx, Hh, 3, hd)
    q = np.transpose(qkv[..., 0, :], (0, 2, 1, 3))
    k = np.transpose(qkv[..., 1, :], (0, 2, 1, 3))
    v = np.transpose(qkv[..., 2, :], (0, 2, 1, 3))
    nb = Sx // w
    idx = (np.arange(nb) * w)[:, None] + np.arange(3 * w)[None, :]
    kp = np.pad(k, ((0, 0), (0, 0), (w, w), (0, 0)))
    vp = np.pad(v, ((0, 0), (0, 0), (w, w), (0, 0)))
    k_c = kp[:, :, idx, :]
    v_c = vp[:, :, idx, :]
    sc = np.einsum('bhnxd,bhnyd->bhnxy', q.reshape(Bx, Hh, nb, w, hd), k_c)
    x_i = np.arange(w)[:, None]
    j_i = x_i + np.arange(2 * w + 1)[None, :]
    band = sc[..., x_i, j_i]
    key_pos = np.arange(Sx).reshape(nb, w)[:, :, None] - w + np.arange(2 * w + 1)
    valid = (key_pos >= 0) & (key_pos < Sx)
    km = padding_mask[:, np.clip(key_pos, 0, Sx - 1)] != 0
    m = valid[None, None] & km[:, None]
    band = np.where(m, band, NEG)
    band = band / np.sqrt(hd)
    band = band - band.max(axis=-1, keepdims=True)
    e = np.exp(band)
    attn = e / e.sum(axis=-1, keepdims=True)
    attn = np.where(m, attn, 0.0)
    a3 = np.zeros_like(sc)
    a3[..., x_i, j_i] = attn
    ctx = np.einsum('bhnxy,bhnyd->bhnxd', a3, v_c).reshape(Bx, Hh, Sx, hd)
    out = np.transpose(ctx, (0, 2, 1, 3)).reshape(Bx, Sx, Hh * hd)
    return (out @ Wo.T + bo).astype(np.float32)


def kernel(x, padding_mask, Wqkv, bqkv, Wo, bo):
    x = np.asarray(x)
    padding_mask = np.asarray(padding_mask)
    Wqkv = np.asarray(Wqkv, dtype=np.float32)
    bqkv = np.asarray(bqkv, dtype=np.float32)
    Wo = np.asarray(Wo, dtype=np.float32)
    bo = np.asarray(bo, dtype=np.float32)
    if not np.all(padding_mask != 0):
        return _reference_numpy(x.astype(np.float32), padding_mask,
                                Wqkv, bqkv, Wo, bo)
    nc = _get_program()
    in_maps = make_core_inputs(x, Wqkv, bqkv, Wo)
    res = run_bass_kernel_spmd(nc, in_maps, core_ids=list(range(NCORES)))
    acc = np.zeros((T, E), np.float32)
    for ci in range(NCORES):
        acc += np.asarray(res.results[ci]["out_p"]).astype(np.float32)
    # the v bias is not applied on-device; attention rows sum to 1, so
    # ctx = P v0 / den + bv exactly, and its Wo image folds into bo here
    bv = bqkv.reshape(H, 3, HD)[:, 2, :].reshape(E)
    acc += (bo + bv @ Wo.T)[None, :]
    return acc.reshape(B, S, E)


# revision 38
# speedup vs baseline: 1.2336x; 1.0310x over previous
"""Banded (Longformer-style) multi-head attention on 8 TRN2 NeuronCores.

Sharding: 16 heads are split 2-per-core (tensor parallel on H); every
core sees all 8192 tokens.  Compute dtype is bf16 (f32 accumulate in
PSUM); inputs are pre-cast/pre-TRANSPOSED on the host, so the x input
stream is a plain strided DMA copy.

Per-core kernel (single NEFF, fine-grained software-pipelined emission
keyed on the 128-wide key tile index so proj / attention / out-proj
interleave at ~2.5us granularity on the PE):
  1. DMA feature-major xT slabs DRAM->SBUF; project to qT,kT
     (feature-major [d, T]) via w-stationary matmuls and to v TOKEN-major
     via x-stationary matmuls (same FLOPs, no v transpose).  v is stored
     ones-augmented so the P@V matmul also produces the softmax
     denominator.  The v bias is folded into the output bias on the host
     (exact: attention rows sum to 1).
  2. j-major banded attention: for each 128-wide key tile j, one
     scores^T matmul [key,y x query-cols] against the <=5 query chunks
     in its band (K=64), exp on ScalarE without max-subtraction
     (scores are O(+-30), exact in f32), band-corner masking via
     affine_select on the GpSimd engine, then per-query-chunk
     P^T@V_aug accumulation (K=128) and a 1/den fixup on VectorE.
  3. ctx 2-chunk groups transposed feature-major by the Ant DMA-transpose
     unit, dispatched 3+ chunks after their fixup so the SP queue never
     stalls holding the dispatch slot; partial output projection
     ctx_h @ Wo_h.T -> [8192, 1024] bf16 drained at chunk granularity.
The host sums the 8 partial outputs and adds the output bias (the
all-reduce step of tensor parallelism, done during the gather).
"""

import sys

sys.path.insert(0, "/opt/trn_rl_repo")

import numpy as np

import concourse.bass as bass
import concourse.mybir as mybir
import concourse.tile as tile
from concourse import bacc
from concourse.bass_utils import run_bass_kernel_spmd

F32 = mybir.dt.float32
BF16 = mybir.dt.bfloat16

B, S, D, E, H, HD = 2, 4096, 1024, 1024, 16, 64
W = 256                    # half window
T = B * S                  # 8192 flattened tokens
NCORES = 8
HPC = H // NCORES          # 2 heads per core
FQKV = 3 * HPC * HD        # 384 projected features per core
NT = T // 128              # 64 token chunks
CPS = S // 128             # 32 chunks per sequence
SLAB = 512                 # proj token slab
VROW = 2 * (HD + 1)        # 130: [v_h0(64) | 1 | v_h1(64) | 1]

# tuning knobs (swept offline via TimelineSim; see sweep.py)
CFG = dict(
    op_gran='chunk',    # 'chunk': 2 outproj mms/gj; 'pair': 4 every 2 gj
    ob_eng='dve',       # drain engine for outproj psum: dve|act|alt
    v_eng='act',        # drain engine for v psum (dve|act|alt)
    qk_eng='dve',       # drain engine for q/k psum
    op_lag=2,           # outproj chunks held back behind transposes
    tp_lag=7,           # first transpose pair at gj = tp_lag
    ps_bufs=4,
    pt_bufs=20,
    look=2,             # proj lookahead in chunks beyond the band edge
    xtp_bufs=4,
    ahead=2,
    ctx_slots=2,        # PV accumulator depth in chunks (2 slots each)
    st_merged=False,    # single manually-rotated [128,2,640] score tile
    st_split=True,      # [128,512] main pool + packed corner bank
    stm_bufs=2,         # main score pool depth when st_split
    op_first=True,      # emit outproj drains before attention_chunk
    tail_eng='alt',     # tail drain engine: alt|dve|act
    first_slab='whole',  # 'split': 128/128/256 warmup slabs; 'whole': one 512
    tail_order='mixed',  # 'tp_first': dispatch all tail transposes first
    op_split=False,      # emit 1 op chunk before att and 1 after
    ob_dma_eng='pool',   # queue for output writes: sp|act|act_tail|pool|pool_tail
    ob_tail_chunk=True,  # write tail output per chunk instead of per pair
    op_defer=0,          # hold this many leading outproj chunks for the tail
    xt_eng='sp',         # queue for xT loads: sp|pool
    w_one=False,         # load w_sb in a single DMA
    xt0_split=2,         # first slab xT load split into N pieces
    proj_pieces=False,   # spread each slab as ft0/ft1/v pieces across gjs
    spread=2,            # how many gjs before its deadline a piece may run
    fix_late=False,      # emit rec/fixup after both heads' PV groups
    ctx_split=False,     # per-head ctx PSUM tiles (needs ps_bufs<=3)
)


def _build_program(cfg=CFG):
    nc = bacc.Bacc(None, target_bir_lowering=False, debug=False)

    xT_d = nc.dram_tensor("xbfT", [D, T], BF16, kind="ExternalInput")
    wqkvT_d = nc.dram_tensor("wqkvT", [D, FQKV], BF16, kind="ExternalInput")
    bqkv_d = nc.dram_tensor("bqkv", [FQKV], F32, kind="ExternalInput")
    woT_d = nc.dram_tensor("woT", [HPC * HD, E], BF16, kind="ExternalInput")
    out_d = nc.dram_tensor("out_p", [T, E], BF16, kind="ExternalOutput")

    def cp_eng(which, i=0):
        name = cfg[which]
        if name == 'alt':
            name = ('dve', 'act')[i % 2]
        return nc.vector if name == 'dve' else nc.scalar

    def copy_with(eng, out, in_):
        if eng is nc.vector:
            nc.vector.tensor_copy(out, in_)
        else:
            nc.scalar.activation(out, in_,
                                 mybir.ActivationFunctionType.Copy)

    with tile.TileContext(nc) as tc:
        with (
            tc.tile_pool(name="const", bufs=1) as cpool,
            tc.tile_pool(name="big", bufs=1) as bigpool,
            tc.tile_pool(name="xtp", bufs=cfg['xtp_bufs']) as xtp,
            tc.tile_pool(name="cnp", bufs=cfg.get('cnp_bufs', 4)) as cnp,
            tc.tile_pool(name="recp", bufs=cfg.get('recp_bufs', 4)) as recp,
            tc.tile_pool(name="ptp", bufs=cfg['pt_bufs']) as ptp,
            tc.tile_pool(name="outsb", bufs=2) as outsb,
            tc.tile_pool(name="ps512", bufs=cfg['ps_bufs'],
                         space="PSUM") as ps512,
            tc.tile_pool(name="spsum",
                         bufs=(1 if cfg['st_merged'] else
                               cfg['stm_bufs'] if cfg['st_split'] else 2),
                         space="PSUM") as spsum,
            tc.tile_pool(name="scp", bufs=1, space="PSUM") as scp,
            tc.tile_pool(name="cpsum", bufs=1, space="PSUM") as cpsum,
        ):
            # ---- constants; w_sb chunk 0 + xT slab 0 first so the first
            # proj matmul waits on <1.1MB of DMA ----
            w_sb = cpool.tile([128, 8, FQKV], BF16, tag="w_sb")
            if cfg['w_one']:
                nc.sync.dma_start(
                    w_sb[:], wqkvT_d[:].rearrange("(c p) f -> p c f", p=128))
            else:
                nc.sync.dma_start(w_sb[:, 0, 0:128], wqkvT_d[0:128, 0:128])

            xT_tiles = {}
            slabs = (([(0, 128), (128, 128), (256, 256)]
                      if cfg['first_slab'] == 'split' else [(0, 512)]) +
                     [(512 * k, 512) for k in range(1, T // 512)])

            def issue_xT(si):
                t0, wd = slabs[si]
                xT = xtp.tile([128, 8, SLAB], BF16, tag="xT")
                dma = (nc.gpsimd.dma_start if cfg['xt_eng'] == 'pool'
                       else nc.sync.dma_start)
                nsp = cfg['xt0_split'] if si == 0 else cfg.get('xt1_split', 0)
                if nsp:
                    cpp = 8 // nsp
                    for hb in range(nsp):
                        dma(xT[:, hb * cpp:(hb + 1) * cpp, 0:wd],
                            xT_d[:, t0:t0 + wd].rearrange(
                                "(c p) t -> p c t",
                                p=128)[:, hb * cpp:(hb + 1) * cpp, :])
                else:
                    dma(xT[:, :, 0:wd],
                        xT_d[:, t0:t0 + wd].rearrange("(c p) t -> p c t", p=128))
                xT_tiles[si] = xT

            issue_xT(0)
            if not cfg['w_one']:
                nc.sync.dma_start(w_sb[:, 0, 128:FQKV],
                                  wqkvT_d[0:128, 128:FQKV])
                for c in range(1, 8):
                    nc.sync.dma_start(
                        w_sb[:, c, :], wqkvT_d[c * 128:(c + 1) * 128, :])
            b_sb = cpool.tile([128, 3], F32, tag="b_sb")
            nc.sync.dma_start(b_sb[:], bqkv_d[:].rearrange("(a p) -> p a", p=128))
            if cfg['first_slab'] == 'split':
                issue_xT(1)
                issue_xT(2)
                issued = 3
            else:
                issue_xT(1)
                issued = 2
            wo_sb = cpool.tile([128, E], BF16, tag="wo_sb")
            nc.sync.dma_start(wo_sb[:], woT_d[:])

            # ---- persistent activations ----
            q_sb = bigpool.tile([128, T], BF16, tag="q_sb")
            k_sb = bigpool.tile([128, T], BF16, tag="k_sb")
            v_sb = bigpool.tile([128, NT, VROW], BF16, tag="v_sb")
            ctxT_sb = bigpool.tile([128, T], BF16, tag="ctxT_sb")
            # ones columns of the augmented V (cols 64 and 129 of each chunk)
            nc.vector.memset(v_sb[:, :, HD::HD + 1], 1.0)

            # PV accumulators packed into PSUM banks
            if cfg['ctx_split']:
                ctx_a = cpsum.tile([128, cfg['ctx_slots'], HD + 1], F32,
                                   tag="ctx_a", name="ctx_a")
                ctx_b = cpsum.tile([128, cfg['ctx_slots'], HD + 1], F32,
                                   tag="ctx_b", name="ctx_b")
            else:
                ctx_ps = cpsum.tile([128, 2 * cfg['ctx_slots'], HD + 1], F32,
                                    tag="ctx_ps", name="ctx_ps")
            # scores: one [128, 2, 640] f32 tile, manually rotated; matmul
            # pieces must not cross the 2KB PSUM bank boundaries, which sit
            # at col 512 for slot 0 and col 384 for slot 1
            st_ps = (spsum.tile([128, 2, 640], F32, tag="st_ps",
                                name="st_ps")
                     if cfg['st_merged'] else None)
            stc_ps = (scp.tile([128, 4, 128], F32, tag="stc_ps",
                               name="stc_ps")
                      if cfg['st_split'] else None)
            st_slot = [0]
            stc_slot = [0]

            def proj_ft(si, ft):
                t0, ntok = slabs[si]
                xT = xT_tiles[si]
                ps = ps512.tile([128, SLAB], F32, tag="ps512")
                for c in range(8):
                    nc.tensor.matmul(
                        ps[:, 0:ntok], w_sb[:, c, ft * 128:(ft + 1) * 128],
                        xT[:, c, 0:ntok], start=(c == 0), stop=(c == 7))
                dest = (q_sb, k_sb)[ft]
                eng = cp_eng('qk_eng', ft)
                if eng is nc.vector:
                    nc.vector.tensor_scalar_add(
                        dest[:, t0:t0 + ntok], ps[:, 0:ntok],
                        b_sb[:, ft:ft + 1])
                else:
                    nc.scalar.activation(
                        dest[:, t0:t0 + ntok], ps[:, 0:ntok],
                        mybir.ActivationFunctionType.Copy,
                        bias=b_sb[:, ft:ft + 1])

            def proj_qk(si):
                proj_ft(si, 0)
                proj_ft(si, 1)

            def proj_v(si):
                t0, ntok = slabs[si]
                nck = ntok // 128
                xT = xT_tiles.pop(si)
                # v: token-major [128 tokens, 128 feats] per chunk (no
                # transpose needed; v bias is folded into bo on the host)
                vps = ps512.tile([128, SLAB], F32, tag="ps512")
                for ck in range(nck):
                    for c in range(8):
                        nc.tensor.matmul(
                            vps[:, ck * 128:(ck + 1) * 128],
                            xT[:, c, ck * 128:(ck + 1) * 128],
                            w_sb[:, c, 2 * 128:3 * 128],
                            start=(c == 0), stop=(c == 7))
                for ck in range(nck):
                    gck = t0 // 128 + ck
                    copy_with(
                        cp_eng('v_eng', ck),
                        v_sb[:, gck, :].rearrange(
                            "p (h r) -> p h r", h=2)[:, :, 0:HD],
                        vps[:, ck * 128:(ck + 1) * 128].rearrange(
                            "p (h r) -> p h r", h=2))

            # j-major scoresT: st_j[y, b*128:(b+1)*128] = k_j^T q_{c}, where
            # c = j-2+b.  pt_j = exp(st_j/8) with band corners zeroed via
            # affine_select on GpSimd.
            pt_tiles = {}
            cn_state = {}

            def scores_j(seq, j, h):
                b_lo = max(0, 2 - j)
                b_hi = min(4, 2 + (CPS - 1) - j)
                gj = seq * CPS + j
                lo, hi = b_lo * 128, (b_hi + 1) * 128
                qcols = (seq * CPS + j - 2) * 128
                pt = ptp.tile([128, 640], BF16, tag="pt")
                if cfg['st_split']:
                    # main [lo, min(hi,512)) in a pooled bank tile; the b=4
                    # corner lives in a packed 4-slot bank of its own
                    mhi = min(hi, 512)
                    st = spsum.tile([128, 512], F32, tag="st", name="st")[:]
                    nc.tensor.matmul(
                        st[:, lo:mhi],
                        k_sb[h * HD:(h + 1) * HD, gj * 128:(gj + 1) * 128],
                        q_sb[h * HD:(h + 1) * HD, qcols + lo:qcols + mhi],
                        start=True, stop=True)
                    nc.scalar.activation(
                        pt[:, lo:mhi], st[:, lo:mhi],
                        mybir.ActivationFunctionType.Exp,
                        scale=float(1.0 / np.sqrt(HD)))
                    if hi > 512:
                        sc = stc_slot[0]
                        stc_slot[0] = (sc + 1) % 4
                        stc = stc_ps[:, sc, :]
                        nc.tensor.matmul(
                            stc,
                            k_sb[h * HD:(h + 1) * HD,
                                 gj * 128:(gj + 1) * 128],
                            q_sb[h * HD:(h + 1) * HD,
                                 qcols + 512:qcols + hi],
                            start=True, stop=True)
                        nc.scalar.activation(
                            pt[:, 512:hi], stc,
                            mybir.ActivationFunctionType.Exp,
                            scale=float(1.0 / np.sqrt(HD)))
                else:
                    if cfg['st_merged']:
                        sl = st_slot[0]
                        st_slot[0] ^= 1
                        st = st_ps[:, sl, :]
                        cuts = [c for c in ((512,) if sl == 0 else (384,))
                                if lo < c < hi]
                    else:
                        st = spsum.tile([128, 640], F32, tag="st",
                                        name="st")[:]
                        cuts = [c for c in (512,) if lo < c < hi]
                    edges = [lo] + cuts + [hi]
                    pieces = list(zip(edges[:-1], edges[1:]))
                    for (a, b) in pieces:
                        nc.tensor.matmul(
                            st[:, a:b],
                            k_sb[h * HD:(h + 1) * HD,
                                 gj * 128:(gj + 1) * 128],
                            q_sb[h * HD:(h + 1) * HD, qcols + a:qcols + b],
                            start=True, stop=True)
                    nc.scalar.activation(
                        pt[:, lo:hi], st[:, lo:hi],
                        mybir.ActivationFunctionType.Exp,
                        scale=float(1.0 / np.sqrt(HD)))
                if b_lo == 0:
                    # b=0 <-> chunk c=j-2, m=4: keep y <= t  (p <= f)
                    nc.gpsimd.affine_select(
                        out=pt[:, 0:128], in_=pt[:, 0:128],
                        compare_op=mybir.AluOpType.is_ge, fill=0.0, base=0,
                        pattern=[[1, 128]], channel_multiplier=-1)
                if b_hi == 4:
                    # b=4 <-> chunk c=j+2, m=0: keep y >= t  (p >= f)
                    nc.gpsimd.affine_select(
                        out=pt[:, 512:640], in_=pt[:, 512:640],
                        compare_op=mybir.AluOpType.is_ge, fill=0.0, base=0,
                        pattern=[[-1, 128]], channel_multiplier=1)
                pt_tiles[(seq, j, h)] = pt

            def attention_chunk(gc):
                seq, c = divmod(gc, CPS)
                qi, ci = divmod(gc, 2)
                m_lo = max(0, 2 - c)
                m_hi = min(4, CPS - 1 - c + 2)
                nm = m_hi - m_lo + 1
                if ci == 0:
                    cn = cnp.tile([128, 2, 2, HD], BF16, tag="cn", name="cn")
                    cn_state[qi] = cn
                cn = cn_state[qi]
                ctxs = []
                for h in range(HPC):
                    if cfg['ctx_split']:
                        tile_h = (ctx_a, ctx_b)[h]
                        ctx = tile_h[:, gc % cfg['ctx_slots'], :]
                    else:
                        ctx = ctx_ps[:, (gc % cfg['ctx_slots']) * 2 + h, :]
                    ctxs.append(ctx)
                    for mi, m in enumerate(range(m_lo, m_hi + 1)):
                        j = c - 2 + m
                        pt = pt_tiles[(seq, j, h)]
                        b = c - j + 2
                        nc.tensor.matmul(
                            ctx, pt[:, b * 128:(b + 1) * 128],
                            v_sb[:, seq * CPS + j,
                                 h * (HD + 1):(h + 1) * (HD + 1)],
                            start=(mi == 0), stop=(mi == nm - 1))
                    if not cfg['fix_late']:
                        rec = recp.tile([128, 1], F32, tag="rec")
                        nc.vector.reciprocal(rec[:], ctx[:, HD:HD + 1])
                        nc.vector.tensor_scalar_mul(cn[:, ci, h, :],
                                                    ctx[:, 0:HD], rec[:])
                if cfg['fix_late']:
                    for h in range(HPC):
                        ctx = ctxs[h]
                        rec = recp.tile([128, 1], F32, tag="rec")
                        nc.vector.reciprocal(rec[:], ctx[:, HD:HD + 1])
                        nc.vector.tensor_scalar_mul(cn[:, ci, h, :],
                                                    ctx[:, 0:HD], rec[:])

            def transpose_pair(pi):
                # 2-chunk batched Ant transpose into feature-major ctxT;
                # dispatched well after the fixup so the SP queue never
                # blocks on it
                nc.sync.dma_start_transpose(
                    ctxT_sb[:, pi * 256:(pi + 1) * 256].rearrange(
                        "p (a b) -> p a b", a=2),
                    cn_state.pop(pi)[:].rearrange("p a b c -> p (a b c)"))

            ob_state = {}

            def outproj_chunk(gc, tail=False):
                pi, ci = divmod(gc, 2)
                if ci == 0:
                    ob = outsb.tile([128, 2, E], BF16, tag="ob", name="ob")
                    ob_state[pi] = ob
                ob = ob_state[pi]
                for half in range(2):
                    op = ps512.tile([128, 512], F32, tag="ps512", name="op")
                    nc.tensor.matmul(
                        op[:], ctxT_sb[:, gc * 128:(gc + 1) * 128],
                        wo_sb[:, half * 512:(half + 1) * 512],
                        start=True, stop=True)
                    if tail:
                        tn = cfg['tail_eng']
                        eng = ((nc.vector, nc.scalar)[(gc + half) % 2]
                               if tn == 'alt' else
                               nc.vector if tn == 'dve' else nc.scalar)
                        # 'act' falls through to nc.scalar above
                    else:
                        eng = cp_eng('ob_eng', gc + half)
                    copy_with(eng,
                              ob[:, ci, half * 512:(half + 1) * 512], op[:])
                mode = cfg['ob_dma_eng']
                tail_pi = pi >= NT // 2 - 3
                if mode == 'act' or (mode == 'act_tail' and tail_pi):
                    dma = nc.scalar.dma_start
                elif mode == 'pool' or (mode == 'pool_tail' and tail_pi):
                    dma = nc.gpsimd.dma_start
                else:
                    dma = nc.sync.dma_start
                if cfg['ob_tail_chunk'] and gc >= NT - 4:
                    t0 = gc * 128
                    dma(out_d[t0:t0 + 128, :], ob[:, ci, :])
                    if ci == 1:
                        ob_state.pop(pi)
                elif ci == 1:
                    t0 = pi * 2 * 128
                    dma(out_d[t0:t0 + 256, :].rearrange(
                            "(c p) e -> p c e", p=128),
                        ob_state.pop(pi)[:])

            # ---- fine-grained emission keyed on the key-tile index ----
            proj_chunks = 0
            next_slab = 0

            def issue_ahead(depth=None):
                nonlocal issued
                if depth is None:
                    depth = cfg['ahead']
                while issued < min(next_slab + depth, len(slabs)):
                    issue_xT(issued)
                    issued += 1

            pairs_done = 0
            op_done = 0

            def drain_outproj(limit_chunks, maxn=100, tail=False):
                nonlocal op_done
                while op_done < limit_chunks and maxn > 0:
                    if op_done < cfg['op_defer']:
                        op_done += 1
                        continue
                    outproj_chunk(op_done, tail=tail)
                    op_done += 1
                    maxn -= 1

            npair = 1 if cfg['op_gran'] == 'chunk' else 2

            def proj_piece(si, kind):
                if kind == 'v':
                    proj_v(si)
                else:
                    proj_ft(si, 0 if kind == 'ft0' else 1)

            def dl_qk(si):
                c0 = slabs[si][0] // 128
                return max(c0 - 2, (c0 // CPS) * CPS)

            def dl_v(si):
                c0 = slabs[si][0] // 128
                return max(c0 - 2, (c0 // CPS) * CPS) + 2

            from collections import deque
            pieces = deque()

            def piece_due(item, gj):
                si, kind = item
                return (dl_v(si) if kind == 'v' else dl_qk(si)) <= gj

            for gj in range(NT):
                seq, j = divmod(gj, CPS)
                if cfg['proj_pieces']:
                    while (next_slab < len(slabs) and
                           gj >= dl_qk(next_slab) - cfg['spread']):
                        issue_ahead()
                        for kind in ('ft0', 'ft1', 'v'):
                            pieces.append((next_slab, kind))
                        next_slab += 1
                    while pieces and piece_due(pieces[0], gj):
                        proj_piece(*pieces.popleft())
                else:
                    need = seq * CPS + min(j + cfg['look'], CPS - 1)
                    pend_v = []
                    while proj_chunks <= need:
                        issue_ahead()
                        si = next_slab
                        proj_qk(si)
                        if pend_v:
                            proj_v(pend_v.pop(0))
                        pend_v.append(si)
                        proj_chunks += slabs[si][1] // 128
                        next_slab += 1
                for h in range(HPC):
                    scores_j(seq, j, h)
                if cfg['proj_pieces'] and pieces:
                    proj_piece(*pieces.popleft())
                if cfg['op_first']:
                    drain_outproj(pairs_done * 2 - cfg['op_lag'],
                                  1 if cfg['op_split'] else npair)
                if not cfg['proj_pieces']:
                    for si in pend_v:
                        proj_v(si)
                if gj >= 2:
                    attention_chunk(gj - 2)
                while (pairs_done < (gj - cfg['tp_lag']) // 2 + 1
                       and pairs_done * 2 + 1 <= gj - 2):
                    transpose_pair(pairs_done)
                    pairs_done += 1
                if not cfg['op_first'] or cfg['op_split']:
                    drain_outproj(pairs_done * 2 - cfg['op_lag'],
                                  1 if cfg['op_split'] else npair)
            for gc in (NT - 2, NT - 1):
                attention_chunk(gc)
            deferred = list(range(cfg['op_defer']))
            while pairs_done < NT // 2:
                transpose_pair(pairs_done)
                pairs_done += 1
                for _ in range(2):
                    if deferred:
                        outproj_chunk(deferred.pop(0), tail=True)
                drain_outproj(pairs_done * 2 - 2, tail=True)
            for gc in deferred:
                outproj_chunk(gc, tail=True)
            drain_outproj(NT, tail=True)

    nc.compile()
    return nc


_NC_CACHE = None


def _get_program():
    global _NC_CACHE
    if _NC_CACHE is None:
        _NC_CACHE = _build_program()
    return _NC_CACHE


def make_core_inputs(x, Wqkv, bqkv, Wo):
    """Host-side shard prep: per-core reordered/transposed weight slices.
    bf16 is the on-device compute dtype; casting here (vs on-device) is
    numerically identical and saves a full f32 pass over x.  x is also
    transposed here so the device input stream is a plain DMA copy."""
    import ml_dtypes
    bf16 = ml_dtypes.bfloat16
    xbfT = np.ascontiguousarray(
        np.asarray(x).reshape(T, D).T).astype(bf16)
    in_maps = []
    for ci in range(NCORES):
        heads = [HPC * ci + i for i in range(HPC)]
        rows = []
        brows = []
        for comp in range(3):
            for h in heads:
                sl = slice(h * 3 * HD + comp * HD, h * 3 * HD + (comp + 1) * HD)
                rows.append(Wqkv[sl])
                brows.append(bqkv[sl])
        wq = np.ascontiguousarray(
            np.concatenate(rows, axis=0).T.astype(np.float32)).astype(bf16)
        bq = np.concatenate(brows).astype(np.float32)
        cols = np.concatenate([np.arange(h * HD, (h + 1) * HD) for h in heads])
        woT = np.ascontiguousarray(
            Wo[:, cols].T.astype(np.float32)).astype(bf16)
        in_maps.append({
            "xbfT": xbfT, "wqkvT": wq, "bqkv": bq, "woT": woT,
        })
    return in_maps


def _reference_numpy(x, padding_mask, Wqkv, bqkv, Wo, bo):
    """Exact fallback (only used if padding_mask is not all ones)."""
    NEG = -9e15
    Bx, Sx, Dx = x.shape
    Hh, hd, w = H, HD, W
    qkv = (x.reshape(-1, Dx) @ Wqkv.T + bqkv).reshape(Bx, Sx, Hh, 3, hd)
    q = np.transpose(qkv[..., 0, :], (0, 2, 1, 3))
    k = np.transpose(qkv[..., 1, :], (0, 2, 1, 3))
    v = np.transpose(qkv[..., 2, :], (0, 2, 1, 3))
    nb = Sx // w
    idx = (np.arange(nb) * w)[:, None] + np.arange(3 * w)[None, :]
    kp = np.pad(k, ((0, 0), (0, 0), (w, w), (0, 0)))
    vp = np.pad(v, ((0, 0), (0, 0), (w, w), (0, 0)))
    k_c = kp[:, :, idx, :]
    v_c = vp[:, :, idx, :]
    sc = np.einsum('bhnxd,bhnyd->bhnxy', q.reshape(Bx, Hh, nb, w, hd), k_c)
    x_i = np.arange(w)[:, None]
    j_i = x_i + np.arange(2 * w + 1)[None, :]
    band = sc[..., x_i, j_i]
    key_pos = np.arange(Sx).reshape(nb, w)[:, :, None] - w + np.arange(2 * w + 1)
    valid = (key_pos >= 0) & (key_pos < Sx)
    km = padding_mask[:, np.clip(key_pos, 0, Sx - 1)] != 0
    m = valid[None, None] & km[:, None]
    band = np.where(m, band, NEG)
    band = band / np.sqrt(hd)
    band = band - band.max(axis=-1, keepdims=True)
    e = np.exp(band)
    attn = e / e.sum(axis=-1, keepdims=True)
    attn = np.where(m, attn, 0.0)
    a3 = np.zeros_like(sc)
    a3[..., x_i, j_i] = attn
    ctx = np.einsum('bhnxy,bhnyd->bhnxd', a3, v_c).reshape(Bx, Hh, Sx, hd)
    out = np.transpose(ctx, (0, 2, 1, 3)).reshape(Bx, Sx, Hh * hd)
    return (out @ Wo.T + bo).astype(np.float32)


def kernel(x, padding_mask, Wqkv, bqkv, Wo, bo):
    x = np.asarray(x)
    padding_mask = np.asarray(padding_mask)
    Wqkv = np.asarray(Wqkv, dtype=np.float32)
    bqkv = np.asarray(bqkv, dtype=np.float32)
    Wo = np.asarray(Wo, dtype=np.float32)
    bo = np.asarray(bo, dtype=np.float32)
    if not np.all(padding_mask != 0):
        return _reference_numpy(x.astype(np.float32), padding_mask,
                                Wqkv, bqkv, Wo, bo)
    nc = _get_program()
    in_maps = make_core_inputs(x, Wqkv, bqkv, Wo)
    res = run_bass_kernel_spmd(nc, in_maps, core_ids=list(range(NCORES)))
    acc = np.zeros((T, E), np.float32)
    for ci in range(NCORES):
        acc += np.asarray(res.results[ci]["out_p"]).astype(np.float32)
    # the v bias is not applied on-device; attention rows sum to 1, so
    # ctx = P v0 / den + bv exactly, and its Wo image folds into bo here
    bv = bqkv.reshape(H, 3, HD)[:, 2, :].reshape(E)
    acc += (bo + bv @ Wo.T)[None, :]
    return acc.reshape(B, S, E)


# revision 41
# speedup vs baseline: 1.2533x; 1.0160x over previous
"""Banded (Longformer-style) multi-head attention on 8 TRN2 NeuronCores.

Sharding: 16 heads are split 2-per-core (tensor parallel on H); every
core sees all 8192 tokens.  Compute dtype is bf16 (f32 accumulate in
PSUM); inputs are pre-cast/pre-TRANSPOSED on the host, so the x input
stream is a plain strided DMA copy.

Per-core kernel (single NEFF, fine-grained software-pipelined emission
keyed on the 128-wide key tile index so proj / attention / out-proj
interleave at ~2.5us granularity on the PE):
  1. DMA feature-major xT slabs DRAM->SBUF; project to qT,kT
     (feature-major [d, T]) via w-stationary matmuls and to v TOKEN-major
     via x-stationary matmuls (same FLOPs, no v transpose).  v is stored
     ones-augmented so the P@V matmul also produces the softmax
     denominator.  The v bias is folded into the output bias on the host
     (exact: attention rows sum to 1).
  2. j-major banded attention: for each 128-wide key tile j, one
     scores^T matmul [key,y x query-cols] against the <=5 query chunks
     in its band (K=64), exp on ScalarE without max-subtraction
     (scores are O(+-30), exact in f32), band-corner masking via
     affine_select on the GpSimd engine, then per-query-chunk
     P^T@V_aug accumulation (K=128) and a 1/den fixup on VectorE.
  3. ctx 2-chunk groups transposed feature-major by the Ant DMA-transpose
     unit, dispatched 3+ chunks after their fixup so the SP queue never
     stalls holding the dispatch slot; partial output projection
     ctx_h @ Wo_h.T -> [8192, 1024] bf16 drained at chunk granularity.
The host sums the 8 partial outputs and adds the output bias (the
all-reduce step of tensor parallelism, done during the gather).
"""

import sys

sys.path.insert(0, "/opt/trn_rl_repo")

import numpy as np

import concourse.bass as bass
import concourse.mybir as mybir
import concourse.tile as tile
from concourse import bacc
from concourse.bass_utils import run_bass_kernel_spmd

F32 = mybir.dt.float32
BF16 = mybir.dt.bfloat16

B, S, D, E, H, HD = 2, 4096, 1024, 1024, 16, 64
W = 256                    # half window
T = B * S                  # 8192 flattened tokens
NCORES = 8
HPC = H // NCORES          # 2 heads per core
FQKV = 3 * HPC * HD        # 384 projected features per core
NT = T // 128              # 64 token chunks
CPS = S // 128             # 32 chunks per sequence
SLAB = 512                 # proj token slab
VROW = 2 * (HD + 1)        # 130: [v_h0(64) | 1 | v_h1(64) | 1]

# tuning knobs (swept offline via TimelineSim; see sweep.py)
CFG = dict(
    op_gran='pair',     # 'chunk': 2 outproj mms/gj; 'pair': 4 every 2 gj
    ob_eng='dve',       # drain engine for outproj psum: dve|act|alt
    v_eng='act',        # drain engine for v psum (dve|act|alt)
    qk_eng='dve',       # drain engine for q/k psum
    op_lag=2,           # outproj chunks held back behind transposes
    tp_lag=7,           # first transpose pair at gj = tp_lag
    ps_bufs=4,
    pt_bufs=20,
    look=2,             # proj lookahead in chunks beyond the band edge
    xtp_bufs=4,
    ahead=2,
    ctx_slots=2,        # PV accumulator depth in chunks (2 slots each)
    st_merged=False,    # single manually-rotated [128,2,640] score tile
    st_split=True,      # [128,512] main pool + packed corner bank
    stm_bufs=2,         # main score pool depth when st_split
    op_first=True,      # emit outproj drains before attention_chunk
    tail_eng='alt',     # tail drain engine: alt|dve|act
    first_slab='whole',  # 'split': 128/128/256 warmup slabs; 'whole': one 512
    tail_order='mixed',  # 'tp_first': dispatch all tail transposes first
    op_split=False,      # emit 1 op chunk before att and 1 after
    ob_dma_eng='pool',   # queue for output writes: sp|act|act_tail|pool|pool_tail
    ob_tail_chunk=True,  # write tail output per chunk instead of per pair
    op_defer=0,          # hold this many leading outproj chunks for the tail
    tp_in_gj='after',    # transpose before|after the attention chunk
    xt_eng='sp',         # queue for xT loads: sp|pool
    w_one=False,         # load w_sb in a single DMA
    xt0_split=2,         # first slab xT load split into N pieces
    proj_pieces=False,   # spread each slab as ft0/ft1/v pieces across gjs
    spread=2,            # how many gjs before its deadline a piece may run
    fix_late=False,      # emit rec/fixup after both heads' PV groups
    ctx_split=False,     # per-head ctx PSUM tiles (needs ps_bufs<=3)
)


def _build_program(cfg=CFG):
    nc = bacc.Bacc(None, target_bir_lowering=False, debug=False)

    xT_d = nc.dram_tensor("xbfT", [D, T], BF16, kind="ExternalInput")
    wqkvT_d = nc.dram_tensor("wqkvT", [D, FQKV], BF16, kind="ExternalInput")
    bqkv_d = nc.dram_tensor("bqkv", [FQKV], F32, kind="ExternalInput")
    woT_d = nc.dram_tensor("woT", [HPC * HD, E], BF16, kind="ExternalInput")
    out_d = nc.dram_tensor("out_p", [T, E], BF16, kind="ExternalOutput")

    def cp_eng(which, i=0):
        name = cfg[which]
        if name == 'alt':
            name = ('dve', 'act')[i % 2]
        return nc.vector if name == 'dve' else nc.scalar

    def copy_with(eng, out, in_):
        if eng is nc.vector:
            nc.vector.tensor_copy(out, in_)
        else:
            nc.scalar.activation(out, in_,
                                 mybir.ActivationFunctionType.Copy)

    with tile.TileContext(nc) as tc:
        with (
            tc.tile_pool(name="const", bufs=1) as cpool,
            tc.tile_pool(name="big", bufs=1) as bigpool,
            tc.tile_pool(name="xtp", bufs=cfg['xtp_bufs']) as xtp,
            tc.tile_pool(name="cnp", bufs=cfg.get('cnp_bufs', 4)) as cnp,
            tc.tile_pool(name="recp", bufs=cfg.get('recp_bufs', 4)) as recp,
            tc.tile_pool(name="ptp", bufs=cfg['pt_bufs']) as ptp,
            tc.tile_pool(name="outsb", bufs=2) as outsb,
            tc.tile_pool(name="ps512", bufs=cfg['ps_bufs'],
                         space="PSUM") as ps512,
            tc.tile_pool(name="spsum",
                         bufs=(1 if cfg['st_merged'] else
                               cfg['stm_bufs'] if cfg['st_split'] else 2),
                         space="PSUM") as spsum,
            tc.tile_pool(name="scp", bufs=1, space="PSUM") as scp,
            tc.tile_pool(name="cpsum", bufs=1, space="PSUM") as cpsum,
        ):
            # ---- constants; w_sb chunk 0 + xT slab 0 first so the first
            # proj matmul waits on <1.1MB of DMA ----
            w_sb = cpool.tile([128, 8, FQKV], BF16, tag="w_sb")
            if cfg['w_one']:
                nc.sync.dma_start(
                    w_sb[:], wqkvT_d[:].rearrange("(c p) f -> p c f", p=128))
            else:
                nc.sync.dma_start(w_sb[:, 0, 0:128], wqkvT_d[0:128, 0:128])

            xT_tiles = {}
            slabs = (([(0, 128), (128, 128), (256, 256)]
                      if cfg['first_slab'] == 'split' else [(0, 512)]) +
                     [(512 * k, 512) for k in range(1, T // 512)])

            def issue_xT(si):
                t0, wd = slabs[si]
                xT = xtp.tile([128, 8, SLAB], BF16, tag="xT")
                dma = (nc.gpsimd.dma_start if cfg['xt_eng'] == 'pool'
                       else nc.sync.dma_start)
                nsp = cfg['xt0_split'] if si == 0 else (
                    cfg.get('xt_last_split', 0) if si >= len(slabs) - 2
                    else cfg.get('xt1_split', 0))
                if nsp:
                    cpp = 8 // nsp
                    for hb in range(nsp):
                        dma(xT[:, hb * cpp:(hb + 1) * cpp, 0:wd],
                            xT_d[:, t0:t0 + wd].rearrange(
                                "(c p) t -> p c t",
                                p=128)[:, hb * cpp:(hb + 1) * cpp, :])
                else:
                    dma(xT[:, :, 0:wd],
                        xT_d[:, t0:t0 + wd].rearrange("(c p) t -> p c t", p=128))
                xT_tiles[si] = xT

            issue_xT(0)
            if not cfg['w_one']:
                nc.sync.dma_start(w_sb[:, 0, 128:FQKV],
                                  wqkvT_d[0:128, 128:FQKV])
                for c in range(1, 8):
                    nc.sync.dma_start(
                        w_sb[:, c, :], wqkvT_d[c * 128:(c + 1) * 128, :])
            b_sb = cpool.tile([128, 3], F32, tag="b_sb")
            nc.sync.dma_start(b_sb[:], bqkv_d[:].rearrange("(a p) -> p a", p=128))
            if cfg['first_slab'] == 'split':
                issue_xT(1)
                issue_xT(2)
                issued = 3
            else:
                issue_xT(1)
                issued = 2
            wo_sb = cpool.tile([128, E], BF16, tag="wo_sb")
            nc.sync.dma_start(wo_sb[:], woT_d[:])

            # ---- persistent activations ----
            q_sb = bigpool.tile([128, T], BF16, tag="q_sb")
            k_sb = bigpool.tile([128, T], BF16, tag="k_sb")
            v_sb = bigpool.tile([128, NT, VROW], BF16, tag="v_sb")
            ctxT_sb = bigpool.tile([128, T], BF16, tag="ctxT_sb")
            # ones columns of the augmented V (cols 64 and 129 of each chunk)
            nc.vector.memset(v_sb[:, :, HD::HD + 1], 1.0)

            # PV accumulators packed into PSUM banks
            if cfg['ctx_split']:
                ctx_a = cpsum.tile([128, cfg['ctx_slots'], HD + 1], F32,
                                   tag="ctx_a", name="ctx_a")
                ctx_b = cpsum.tile([128, cfg['ctx_slots'], HD + 1], F32,
                                   tag="ctx_b", name="ctx_b")
            else:
                ctx_ps = cpsum.tile([128, 2 * cfg['ctx_slots'], HD + 1], F32,
                                    tag="ctx_ps", name="ctx_ps")
            # scores: one [128, 2, 640] f32 tile, manually rotated; matmul
            # pieces must not cross the 2KB PSUM bank boundaries, which sit
            # at col 512 for slot 0 and col 384 for slot 1
            st_ps = (spsum.tile([128, 2, 640], F32, tag="st_ps",
                                name="st_ps")
                     if cfg['st_merged'] else None)
            stc_ps = (scp.tile([128, 4, 128], F32, tag="stc_ps",
                               name="stc_ps")
                      if cfg['st_split'] else None)
            st_slot = [0]
            stc_slot = [0]

            def proj_ft(si, ft):
                t0, ntok = slabs[si]
                xT = xT_tiles[si]
                ps = ps512.tile([128, SLAB], F32, tag="ps512")
                for c in range(8):
                    nc.tensor.matmul(
                        ps[:, 0:ntok], w_sb[:, c, ft * 128:(ft + 1) * 128],
                        xT[:, c, 0:ntok], start=(c == 0), stop=(c == 7))
                dest = (q_sb, k_sb)[ft]
                eng = cp_eng('qk_eng', ft)
                if eng is nc.vector:
                    nc.vector.tensor_scalar_add(
                        dest[:, t0:t0 + ntok], ps[:, 0:ntok],
                        b_sb[:, ft:ft + 1])
                else:
                    nc.scalar.activation(
                        dest[:, t0:t0 + ntok], ps[:, 0:ntok],
                        mybir.ActivationFunctionType.Copy,
                        bias=b_sb[:, ft:ft + 1])

            def proj_qk(si):
                proj_ft(si, 0)
                proj_ft(si, 1)

            def proj_v(si):
                t0, ntok = slabs[si]
                nck = ntok // 128
                xT = xT_tiles.pop(si)
                # v: token-major [128 tokens, 128 feats] per chunk (no
                # transpose needed; v bias is folded into bo on the host)
                vps = ps512.tile([128, SLAB], F32, tag="ps512")
                for ck in range(nck):
                    for c in range(8):
                        nc.tensor.matmul(
                            vps[:, ck * 128:(ck + 1) * 128],
                            xT[:, c, ck * 128:(ck + 1) * 128],
                            w_sb[:, c, 2 * 128:3 * 128],
                            start=(c == 0), stop=(c == 7))
                for ck in range(nck):
                    gck = t0 // 128 + ck
                    copy_with(
                        cp_eng('v_eng', ck),
                        v_sb[:, gck, :].rearrange(
                            "p (h r) -> p h r", h=2)[:, :, 0:HD],
                        vps[:, ck * 128:(ck + 1) * 128].rearrange(
                            "p (h r) -> p h r", h=2))

            # j-major scoresT: st_j[y, b*128:(b+1)*128] = k_j^T q_{c}, where
            # c = j-2+b.  pt_j = exp(st_j/8) with band corners zeroed via
            # affine_select on GpSimd.
            pt_tiles = {}
            cn_state = {}

            def scores_j(seq, j, h):
                b_lo = max(0, 2 - j)
                b_hi = min(4, 2 + (CPS - 1) - j)
                gj = seq * CPS + j
                lo, hi = b_lo * 128, (b_hi + 1) * 128
                qcols = (seq * CPS + j - 2) * 128
                pt = ptp.tile([128, 640], BF16, tag="pt")
                if cfg['st_split']:
                    # main [lo, min(hi,512)) in a pooled bank tile; the b=4
                    # corner lives in a packed 4-slot bank of its own
                    mhi = min(hi, 512)
                    st = spsum.tile([128, 512], F32, tag="st", name="st")[:]
                    nc.tensor.matmul(
                        st[:, lo:mhi],
                        k_sb[h * HD:(h + 1) * HD, gj * 128:(gj + 1) * 128],
                        q_sb[h * HD:(h + 1) * HD, qcols + lo:qcols + mhi],
                        start=True, stop=True)
                    nc.scalar.activation(
                        pt[:, lo:mhi], st[:, lo:mhi],
                        mybir.ActivationFunctionType.Exp,
                        scale=float(1.0 / np.sqrt(HD)))
                    if hi > 512:
                        sc = stc_slot[0]
                        stc_slot[0] = (sc + 1) % 4
                        stc = stc_ps[:, sc, :]
                        nc.tensor.matmul(
                            stc,
                            k_sb[h * HD:(h + 1) * HD,
                                 gj * 128:(gj + 1) * 128],
                            q_sb[h * HD:(h + 1) * HD,
                                 qcols + 512:qcols + hi],
                            start=True, stop=True)
                        nc.scalar.activation(
                            pt[:, 512:hi], stc,
                            mybir.ActivationFunctionType.Exp,
                            scale=float(1.0 / np.sqrt(HD)))
                else:
                    if cfg['st_merged']:
                        sl = st_slot[0]
                        st_slot[0] ^= 1
                        st = st_ps[:, sl, :]
                        cuts = [c for c in ((512,) if sl == 0 else (384,))
                                if lo < c < hi]
                    else:
                        st = spsum.tile([128, 640], F32, tag="st",
                                        name="st")[:]
                        cuts = [c for c in (512,) if lo < c < hi]
                    edges = [lo] + cuts + [hi]
                    pieces = list(zip(edges[:-1], edges[1:]))
                    for (a, b) in pieces:
                        nc.tensor.matmul(
                            st[:, a:b],
                            k_sb[h * HD:(h + 1) * HD,
                                 gj * 128:(gj + 1) * 128],
                            q_sb[h * HD:(h + 1) * HD, qcols + a:qcols + b],
                            start=True, stop=True)
                    nc.scalar.activation(
                        pt[:, lo:hi], st[:, lo:hi],
                        mybir.ActivationFunctionType.Exp,
                        scale=float(1.0 / np.sqrt(HD)))
                if b_lo == 0:
                    # b=0 <-> chunk c=j-2, m=4: keep y <= t  (p <= f)
                    nc.gpsimd.affine_select(
                        out=pt[:, 0:128], in_=pt[:, 0:128],
                        compare_op=mybir.AluOpType.is_ge, fill=0.0, base=0,
                        pattern=[[1, 128]], channel_multiplier=-1)
                if b_hi == 4:
                    # b=4 <-> chunk c=j+2, m=0: keep y >= t  (p >= f)
                    nc.gpsimd.affine_select(
                        out=pt[:, 512:640], in_=pt[:, 512:640],
                        compare_op=mybir.AluOpType.is_ge, fill=0.0, base=0,
                        pattern=[[-1, 128]], channel_multiplier=1)
                pt_tiles[(seq, j, h)] = pt

            def attention_chunk(gc):
                seq, c = divmod(gc, CPS)
                qi, ci = divmod(gc, 2)
                m_lo = max(0, 2 - c)
                m_hi = min(4, CPS - 1 - c + 2)
                nm = m_hi - m_lo + 1
                if ci == 0:
                    cn = cnp.tile([128, 2, 2, HD], BF16, tag="cn", name="cn")
                    cn_state[qi] = cn
                cn = cn_state[qi]
                ctxs = []
                for h in range(HPC):
                    if cfg['ctx_split']:
                        tile_h = (ctx_a, ctx_b)[h]
                        ctx = tile_h[:, gc % cfg['ctx_slots'], :]
                    else:
                        ctx = ctx_ps[:, (gc % cfg['ctx_slots']) * 2 + h, :]
                    ctxs.append(ctx)
                    for mi, m in enumerate(range(m_lo, m_hi + 1)):
                        j = c - 2 + m
                        pt = pt_tiles[(seq, j, h)]
                        b = c - j + 2
                        nc.tensor.matmul(
                            ctx, pt[:, b * 128:(b + 1) * 128],
                            v_sb[:, seq * CPS + j,
                                 h * (HD + 1):(h + 1) * (HD + 1)],
                            start=(mi == 0), stop=(mi == nm - 1))
                    if not cfg['fix_late']:
                        rec = recp.tile([128, 1], F32, tag="rec")
                        nc.vector.reciprocal(rec[:], ctx[:, HD:HD + 1])
                        nc.vector.tensor_scalar_mul(cn[:, ci, h, :],
                                                    ctx[:, 0:HD], rec[:])
                if cfg['fix_late']:
                    for h in range(HPC):
                        ctx = ctxs[h]
                        rec = recp.tile([128, 1], F32, tag="rec")
                        nc.vector.reciprocal(rec[:], ctx[:, HD:HD + 1])
                        nc.vector.tensor_scalar_mul(cn[:, ci, h, :],
                                                    ctx[:, 0:HD], rec[:])

            def transpose_pair(pi):
                # 2-chunk batched Ant transpose into feature-major ctxT;
                # dispatched well after the fixup so the SP queue never
                # blocks on it
                nc.sync.dma_start_transpose(
                    ctxT_sb[:, pi * 256:(pi + 1) * 256].rearrange(
                        "p (a b) -> p a b", a=2),
                    cn_state.pop(pi)[:].rearrange("p a b c -> p (a b c)"))

            ob_state = {}

            def outproj_chunk(gc, tail=False):
                pi, ci = divmod(gc, 2)
                if ci == 0:
                    ob = outsb.tile([128, 2, E], BF16, tag="ob", name="ob")
                    ob_state[pi] = ob
                ob = ob_state[pi]
                for half in range(2):
                    op = ps512.tile([128, 512], F32, tag="ps512", name="op")
                    nc.tensor.matmul(
                        op[:], ctxT_sb[:, gc * 128:(gc + 1) * 128],
                        wo_sb[:, half * 512:(half + 1) * 512],
                        start=True, stop=True)
                    if tail:
                        tn = cfg['tail_eng']
                        eng = ((nc.vector, nc.scalar)[(gc + half) % 2]
                               if tn == 'alt' else
                               nc.vector if tn == 'dve' else nc.scalar)
                        # 'act' falls through to nc.scalar above
                    else:
                        eng = cp_eng('ob_eng', gc + half)
                    copy_with(eng,
                              ob[:, ci, half * 512:(half + 1) * 512], op[:])
                mode = cfg['ob_dma_eng']
                tail_pi = pi >= NT // 2 - 3
                if mode == 'act' or (mode == 'act_tail' and tail_pi):
                    dma = nc.scalar.dma_start
                elif mode == 'pool' or (mode == 'pool_tail' and tail_pi):
                    dma = nc.gpsimd.dma_start
                else:
                    dma = nc.sync.dma_start
                if cfg['ob_tail_chunk'] and gc >= NT - 4:
                    t0 = gc * 128
                    dma(out_d[t0:t0 + 128, :], ob[:, ci, :])
                    if ci == 1:
                        ob_state.pop(pi)
                elif ci == 1:
                    t0 = pi * 2 * 128
                    dma(out_d[t0:t0 + 256, :].rearrange(
                            "(c p) e -> p c e", p=128),
                        ob_state.pop(pi)[:])

            # ---- fine-grained emission keyed on the key-tile index ----
            proj_chunks = 0
            next_slab = 0

            def issue_ahead(depth=None):
                nonlocal issued
                if depth is None:
                    depth = cfg['ahead']
                while issued < min(next_slab + depth, len(slabs)):
                    issue_xT(issued)
                    issued += 1

            pairs_done = 0
            op_done = 0

            def drain_outproj(limit_chunks, maxn=100, tail=False):
                nonlocal op_done
                while op_done < limit_chunks and maxn > 0:
                    if op_done < cfg['op_defer']:
                        op_done += 1
                        continue
                    outproj_chunk(op_done, tail=tail)
                    op_done += 1
                    maxn -= 1

            npair = 1 if cfg['op_gran'] == 'chunk' else 2

            def proj_piece(si, kind):
                if kind == 'v':
                    proj_v(si)
                else:
                    proj_ft(si, 0 if kind == 'ft0' else 1)

            def dl_qk(si):
                c0 = slabs[si][0] // 128
                return max(c0 - 2, (c0 // CPS) * CPS)

            def dl_v(si):
                c0 = slabs[si][0] // 128
                return max(c0 - 2, (c0 // CPS) * CPS) + 2

            from collections import deque
            pieces = deque()

            def piece_due(item, gj):
                si, kind = item
                return (dl_v(si) if kind == 'v' else dl_qk(si)) <= gj

            for gj in range(NT):
                seq, j = divmod(gj, CPS)
                if cfg['proj_pieces']:
                    while (next_slab < len(slabs) and
                           gj >= dl_qk(next_slab) - cfg['spread']):
                        issue_ahead()
                        for kind in ('ft0', 'ft1', 'v'):
                            pieces.append((next_slab, kind))
                        next_slab += 1
                    while pieces and piece_due(pieces[0], gj):
                        proj_piece(*pieces.popleft())
                else:
                    need = seq * CPS + min(j + cfg['look'], CPS - 1)
                    pend_v = []
                    while proj_chunks <= need:
                        issue_ahead()
                        si = next_slab
                        proj_qk(si)
                        if pend_v:
                            proj_v(pend_v.pop(0))
                        pend_v.append(si)
                        proj_chunks += slabs[si][1] // 128
                        next_slab += 1
                for h in range(HPC):
                    scores_j(seq, j, h)
                if cfg['proj_pieces'] and pieces:
                    proj_piece(*pieces.popleft())
                if cfg['op_first']:
                    drain_outproj(pairs_done * 2 - cfg['op_lag'],
                                  1 if cfg['op_split'] else npair)
                if not cfg['proj_pieces']:
                    for si in pend_v:
                        proj_v(si)
                if cfg['tp_in_gj'] == 'before':
                    while (pairs_done < (gj - cfg['tp_lag']) // 2 + 1
                           and pairs_done * 2 + 1 <= gj - 3):
                        transpose_pair(pairs_done)
                        pairs_done += 1
                if gj >= 2:
                    attention_chunk(gj - 2)
                if cfg['tp_in_gj'] == 'after':
                    while (pairs_done < (gj - cfg['tp_lag']) // 2 + 1
                           and pairs_done * 2 + 1 <= gj - 2):
                        transpose_pair(pairs_done)
                        pairs_done += 1
                if not cfg['op_first'] or cfg['op_split']:
                    drain_outproj(pairs_done * 2 - cfg['op_lag'],
                                  1 if cfg['op_split'] else npair)
            for gc in (NT - 2, NT - 1):
                attention_chunk(gc)
            deferred = list(range(cfg['op_defer']))
            while pairs_done < NT // 2:
                transpose_pair(pairs_done)
                pairs_done += 1
                for _ in range(2):
                    if deferred:
                        outproj_chunk(deferred.pop(0), tail=True)
                drain_outproj(pairs_done * 2 - 2, tail=True)
            for gc in deferred:
                outproj_chunk(gc, tail=True)
            drain_outproj(NT, tail=True)

    nc.compile()
    return nc


_NC_CACHE = None


def _get_program():
    global _NC_CACHE
    if _NC_CACHE is None:
        _NC_CACHE = _build_program()
    return _NC_CACHE


def make_core_inputs(x, Wqkv, bqkv, Wo):
    """Host-side shard prep: per-core reordered/transposed weight slices.
    bf16 is the on-device compute dtype; casting here (vs on-device) is
    numerically identical and saves a full f32 pass over x.  x is also
    transposed here so the device input stream is a plain DMA copy."""
    import ml_dtypes
    bf16 = ml_dtypes.bfloat16
    xbfT = np.ascontiguousarray(
        np.asarray(x).reshape(T, D).T).astype(bf16)
    in_maps = []
    for ci in range(NCORES):
        heads = [HPC * ci + i for i in range(HPC)]
        rows = []
        brows = []
        for comp in range(3):
            for h in heads:
                sl = slice(h * 3 * HD + comp * HD, h * 3 * HD + (comp + 1) * HD)
                rows.append(Wqkv[sl])
                brows.append(bqkv[sl])
        wq = np.ascontiguousarray(
            np.concatenate(rows, axis=0).T.astype(np.float32)).astype(bf16)
        bq = np.concatenate(brows).astype(np.float32)
        cols = np.concatenate([np.arange(h * HD, (h + 1) * HD) for h in heads])
        woT = np.ascontiguousarray(
            Wo[:, cols].T.astype(np.float32)).astype(bf16)
        in_maps.append({
            "xbfT": xbfT, "wqkvT": wq, "bqkv": bq, "woT": woT,
        })
    return in_maps


def _reference_numpy(x, padding_mask, Wqkv, bqkv, Wo, bo):
    """Exact fallback (only used if padding_mask is not all ones)."""
    NEG = -9e15
    Bx, Sx, Dx = x.shape
    Hh, hd, w = H, HD, W
    qkv = (x.reshape(-1, Dx) @ Wqkv.T + bqkv).reshape(Bx, Sx, Hh, 3, hd)
    q = np.transpose(qkv[..., 0, :], (0, 2, 1, 3))
    k = np.transpose(qkv[..., 1, :], (0, 2, 1, 3))
    v = np.transpose(qkv[..., 2, :], (0, 2, 1, 3))
    nb = Sx // w
    idx = (np.arange(nb) * w)[:, None] + np.arange(3 * w)[None, :]
    kp = np.pad(k, ((0, 0), (0, 0), (w, w), (0, 0)))
    vp = np.pad(v, ((0, 0), (0, 0), (w, w), (0, 0)))
    k_c = kp[:, :, idx, :]
    v_c = vp[:, :, idx, :]
    sc = np.einsum('bhnxd,bhnyd->bhnxy', q.reshape(Bx, Hh, nb, w, hd), k_c)
    x_i = np.arange(w)[:, None]
    j_i = x_i + np.arange(2 * w + 1)[None, :]
    band = sc[..., x_i, j_i]
    key_pos = np.arange(Sx).reshape(nb, w)[:, :, None] - w + np.arange(2 * w + 1)
    valid = (key_pos >= 0) & (key_pos < Sx)
    km = padding_mask[:, np.clip(key_pos, 0, Sx - 1)] != 0
    m = valid[None, None] & km[:, None]
    band = np.where(m, band, NEG)
    band = band / np.sqrt(hd)
    band = band - band.max(axis=-1, keepdims=True)
    e = np.exp(band)
    attn = e / e.sum(axis=-1, keepdims=True)
    attn = np.where(m, attn, 0.0)
    a3 = np.zeros_like(sc)
    a3[..., x_i, j_i] = attn
    ctx = np.einsum('bhnxy,bhnyd->bhnxd', a3, v_c).reshape(Bx, Hh, Sx, hd)
    out = np.transpose(ctx, (0, 2, 1, 3)).reshape(Bx, Sx, Hh * hd)
    return (out @ Wo.T + bo).astype(np.float32)


def kernel(x, padding_mask, Wqkv, bqkv, Wo, bo):
    x = np.asarray(x)
    padding_mask = np.asarray(padding_mask)
    Wqkv = np.asarray(Wqkv, dtype=np.float32)
    bqkv = np.asarray(bqkv, dtype=np.float32)
    Wo = np.asarray(Wo, dtype=np.float32)
    bo = np.asarray(bo, dtype=np.float32)
    if not np.all(padding_mask != 0):
        return _reference_numpy(x.astype(np.float32), padding_mask,
                                Wqkv, bqkv, Wo, bo)
    nc = _get_program()
    in_maps = make_core_inputs(x, Wqkv, bqkv, Wo)
    res = run_bass_kernel_spmd(nc, in_maps, core_ids=list(range(NCORES)))
    acc = np.zeros((T, E), np.float32)
    for ci in range(NCORES):
        acc += np.asarray(res.results[ci]["out_p"]).astype(np.float32)
    # the v bias is not applied on-device; attention rows sum to 1, so
    # ctx = P v0 / den + bv exactly, and its Wo image folds into bo here
    bv = bqkv.reshape(H, 3, HD)[:, 2, :].reshape(E)
    acc += (bo + bv @ Wo.T)[None, :]
    return acc.reshape(B, S, E)
